# revision 14
# baseline (speedup 1.0000x reference)
"""Periodic radius-graph KNN (minimum-image, K=32) on 8 Trainium2 cores.

Strategy (data-parallel neighbor-list build):
  * Host: sort atoms by spatial cell (8x8x8 grid of 6.25 A cells, box 50 A);
    split the sorted order into blocks of 8 atoms; for each block build a
    candidate list (atoms within cutoff of the block's bounding box, found
    via the cell grid with periodic wrap).  Blocks are then permuted so
    that blocks with similar candidate counts land in the same tile slot
    on every core -- each of the 8 tile slots gets its own compile-time
    candidate capacity C_t (the per-slot max), so DVE/DMA work scales with
    the mean candidate count instead of the global max.
  * Device (SPMD over 8 cores, 1024 atoms each): for every 128-row tile,
    broadcast each 8-row block's candidate coordinate planes across its
    partitions (DMA with partition-broadcast access pattern), compute the
    exact minimum-image squared distance
        d2 = ((mx^2 + my^2) + mz^2),  m_c = min(|dx_c|, 50 - |dx_c|)
    bit-identical to the fp32 reference, and extract the 32 smallest d2
    (with candidate indices) per row via 4 rounds of the DVE max8 /
    max_index / match_replace instructions on the negated keys.
  * Host: map candidate slots back to atom indices, apply the cutoff,
    restore original atom order, and assemble edge_index / edge_weight /
    edge_vec exactly as the reference does.

The fp32 identity min(|dx|, 50-|dx|) reproduces jnp's
`diff - round(diff/box)*box` bit-exactly for box=50 and coords in [0, 50):
the winning branch of the min is always exactly representable, so the
selection and ordering of neighbors match the reference to the last ulp.
"""

from contextlib import ExitStack

import numpy as np

import concourse.bass as bass
import concourse.tile as tile
from concourse import bacc, mybir
from concourse.bass_utils import run_bass_kernel_spmd

N_CORES = 8
K = 32
BOX = 50.0
CELL = 6.25           # 8 cells per dimension
GRID = 8
CUTOFF2 = np.float32(36.0)
BLK = 8               # rows per candidate block
P = 128               # partitions per tile
PAD_COORD = np.float32(1.0e3)   # padding slot coordinate -> d2 ~ 2.7e6 >> 36


# ----------------------------------------------------------------------------
# host-side preprocessing
# ----------------------------------------------------------------------------

def _build_blocks(pos):
    """Sort atoms by cell; per-BLK-row-block candidate index lists."""
    n = pos.shape[0]
    cell = np.minimum((pos // np.float32(CELL)).astype(np.int64), GRID - 1)
    cid = (cell[:, 0] * GRID + cell[:, 1]) * GRID + cell[:, 2]
    order = np.argsort(cid, kind="stable").astype(np.int64)
    pos_s = pos[order]
    cid_s = cid[order]

    atoms_by_cell = [[] for _ in range(GRID ** 3)]
    for i, c in enumerate(cid):
        atoms_by_cell[c].append(i)
    atoms_by_cell = [np.asarray(a, dtype=np.int64) for a in atoms_by_cell]

    n_blocks = n // BLK
    posd = pos.astype(np.float64)
    cand_lists = []
    for blk in range(n_blocks):
        rows = pos_s[blk * BLK:(blk + 1) * BLK].astype(np.float64)
        cells = np.unique(cid_s[blk * BLK:(blk + 1) * BLK])
        seen = set()
        for cc in cells:
            ca, rem = divmod(int(cc), GRID * GRID)
            cb, ccz = divmod(rem, GRID)
            for da in (-1, 0, 1):
                for db in (-1, 0, 1):
                    for dc in (-1, 0, 1):
                        seen.add(
                            (((ca + da) % GRID) * GRID + ((cb + db) % GRID)) * GRID
                            + ((ccz + dc) % GRID))
        seen = [c for c in sorted(seen) if len(atoms_by_cell[c])]
        cand = (np.concatenate([atoms_by_cell[c] for c in seen])
                if seen else np.empty(0, np.int64))
        # exact periodic distance from candidate to block bounding box
        lo = rows.min(axis=0)
        hi = rows.max(axis=0)
        p = posd[cand]
        d = np.zeros(len(cand))
        for k in range(3):
            best = None
            for sh in (-BOX, 0.0, BOX):
                x = p[:, k] + sh
                dd = np.abs(x - np.clip(x, lo[k], hi[k]))
                best = dd if best is None else np.minimum(best, dd)
            d += best * best
        keep = cand[d <= 36.0 + 1e-3]
        cand_lists.append(keep.astype(np.int64))
    return order, cand_lists


def _plan(pos):
    """Full host plan: row permutation, per-slot capacities, DRAM arrays."""
    n = pos.shape[0]
    order0, cand_lists = _build_blocks(pos)
    n_blocks = len(cand_lists)
    ntile = n // (N_CORES * P)
    blk_per_tile = P // BLK
    blocks_per_slot = N_CORES * blk_per_tile

    counts = np.array([len(c) for c in cand_lists])
    rank = np.argsort(counts, kind="stable")       # ascending candidate count
    # Group ranked blocks into ntile slots, then order the slots
    # small, large, small, large, ... so the DMA/ACT pipeline stays ahead of
    # the DVE (a cheap first tile minimises warm-up; alternating sizes avoids
    # a tail stall where the DVE catches up with the prefetch of the most
    # expensive tiles).
    grps = [rank[s * blocks_per_slot:(s + 1) * blocks_per_slot]
            for s in range(ntile)]
    interleave = []
    lo, hi = 0, ntile - 1
    while lo <= hi:
        interleave.append(lo)
        if hi != lo:
            interleave.append(hi)
        lo += 1
        hi -= 1
    grps = [grps[i] for i in interleave]
    c_caps = []
    assign = np.empty((N_CORES, ntile, blk_per_tile), dtype=np.int64)
    for s, grp in enumerate(grps):
        cmax = max(40, int(counts[grp].max()))
        c_caps.append(-(-cmax // 8) * 8)
        assign[:, s, :] = grp.reshape(N_CORES, blk_per_tile)

    # new row order: core-major, then slot, then block, then row-in-block
    block_rows = np.arange(n).reshape(n_blocks, BLK)
    new_rows = block_rows[assign.reshape(-1)].reshape(-1)   # sorted-row indices
    order = order0[new_rows]                                # original atom ids

    pos_s = pos[order]
    # packed per-core qneg: [core][128, 3*ntile]  (col 3t+c = -coord c, tile t)
    qneg = np.empty((N_CORES, P, 3 * ntile), dtype=np.float32)
    for kcore in range(N_CORES):
        slab = -pos_s[kcore * ntile * P:(kcore + 1) * ntile * P]
        for t in range(ntile):
            qneg[kcore, :, 3 * t:3 * t + 3] = slab[t * P:(t + 1) * P]

    # per-slot candidate planes + (host-only) index tables
    cands = []          # list over slots: [N_CORES, blk_per_tile, 3, C_s] f32
    for s in range(ntile):
        C = c_caps[s]
        cx = np.full((N_CORES, blk_per_tile, 3, C), PAD_COORD, dtype=np.float32)
        for kcore in range(N_CORES):
            for b in range(blk_per_tile):
                cl = cand_lists[assign[kcore, s, b]]
                cx[kcore, b, :, :len(cl)] = pos[cl].T
        cands.append(cx)

    cand_idx = np.full((n_blocks, max(c_caps)), -1, dtype=np.int64)
    for b, cl in enumerate(cand_lists):
        cand_idx[b, :len(cl)] = cl
    # block id (in original cell-sorted block numbering) for each new row
    row_block = assign.reshape(-1).repeat(BLK)
    return order, tuple(c_caps), qneg, cands, cand_idx, row_block


# ----------------------------------------------------------------------------
# device kernel (built once per (rows_per_core, c_caps) shape)
# ----------------------------------------------------------------------------

def _build_nc(rows_per_core, c_caps):
    ntile = rows_per_core // P
    blk_per_tile = P // BLK
    assert len(c_caps) == ntile

    nc = bacc.Bacc("TRN2", target_bir_lowering=False, debug=False,
                   enable_asserts=False, num_devices=N_CORES)
    qneg_d = nc.dram_tensor("qneg", [P, 3 * ntile], mybir.dt.float32,
                            kind="ExternalInput").ap()
    cand_d = [nc.dram_tensor(f"cand{t}", [blk_per_tile, 3, c_caps[t]],
                             mybir.dt.float32, kind="ExternalInput").ap()
              for t in range(ntile)]
    vals_d = nc.dram_tensor("vals", [rows_per_core, K], mybir.dt.float32,
                            kind="ExternalOutput").ap()
    idxs_d = nc.dram_tensor("idxs", [rows_per_core, K], mybir.dt.uint32,
                            kind="ExternalOutput").ap()

    with tile.TileContext(nc) as tc, ExitStack() as ctx:
        pool = ctx.enter_context(tc.tile_pool(name="work", bufs=3))
        opool = ctx.enter_context(tc.tile_pool(name="outs", bufs=ntile))
        cpool = ctx.enter_context(tc.tile_pool(name="consts", bufs=1))
        out_tiles = []

        bias_m50 = cpool.tile([P, 1], mybir.dt.float32)
        nc.gpsimd.memset(bias_m50[:], -50.0)
        bias_0 = cpool.tile([P, 1], mybir.dt.float32)
        nc.gpsimd.memset(bias_0[:], 0.0)
        qneg = cpool.tile([P, 3 * ntile], mybir.dt.float32)
        nc.sync.dma_start(qneg[:], qneg_d[:])

        for t in range(ntile):
            C = c_caps[t]
            xj = pool.tile([P, 3 * C], mybir.dt.float32, tag="xj")
            for b in range(blk_per_tile):
                nc.sync.dma_start(
                    xj[BLK * b:BLK * (b + 1), :],
                    cand_d[t][b].flatten().partition_broadcast(BLK))

            # adx = |xj - xi| ; exact fp32 (fma single-rounding == plain sub)
            adx = pool.tile([P, 3 * C], mybir.dt.float32, tag="adx")
            for c in range(3):
                nc.scalar.activation(
                    adx[:, C * c:C * (c + 1)], xj[:, C * c:C * (c + 1)],
                    mybir.ActivationFunctionType.Abs,
                    bias=qneg[:, 3 * t + c:3 * t + c + 1], scale=1.0)

            # msq_c = min(adx^2, (adx-50)^2) == wrapped_diff^2, exactly
            sq1 = pool.tile([P, 3 * C], mybir.dt.float32, tag="xj")  # reuse xj buf
            nc.scalar.activation(sq1[:], adx[:],
                                 mybir.ActivationFunctionType.Square,
                                 bias=bias_0[:])
            sq2 = pool.tile([P, 3 * C], mybir.dt.float32, tag="sq2")
            nc.scalar.activation(sq2[:], adx[:],
                                 mybir.ActivationFunctionType.Square,
                                 bias=bias_m50[:])
            msq = sq2
            nc.vector.tensor_tensor(msq[:], sq1[:], sq2[:], mybir.AluOpType.min)

            # key = -d2 = (-msq_x - msq_y) - msq_z  (negated left-to-right sum)
            key = pool.tile([P, C], mybir.dt.float32, tag="key")
            nc.vector.scalar_tensor_tensor(
                key[:], msq[:, 0:C], -1.0, msq[:, C:2 * C],
                mybir.AluOpType.mult, mybir.AluOpType.subtract)
            nc.vector.tensor_tensor(key[:], key[:], msq[:, 2 * C:3 * C],
                                    mybir.AluOpType.subtract)

            # 4 rounds of 8-way extraction = top-32 ascending d2
            out = opool.tile([P, 2 * K], mybir.dt.uint32, tag="out")
            out_tiles.append(out)
            vals = out[:, 0:K].bitcast(mybir.dt.float32)
            idxs = out[:, K:2 * K]
            for r in range(K // 8):
                v8 = vals[:, 8 * r:8 * (r + 1)]
                nc.vector.max(v8, key[:])
                nc.vector.max_index(idxs[:, 8 * r:8 * (r + 1)], v8, key[:])
                if r != K // 8 - 1:
                    nc.vector.match_replace(key[:], v8, key[:], -3.0e38)

        # all output DMAs at the end on the otherwise-idle gpsimd queue, so no
        # engine's instruction stream ever blocks a later tile's work on them
        for t, out in enumerate(out_tiles):
            nc.gpsimd.dma_start(vals_d[t * P:(t + 1) * P, :],
                                out[:, 0:K].bitcast(mybir.dt.float32))
            nc.gpsimd.dma_start(idxs_d[t * P:(t + 1) * P, :], out[:, K:2 * K])
    nc.compile()
    return nc


_NC_CACHE = {}


def _get_nc(rows_per_core, c_caps):
    key = (rows_per_core, c_caps)
    if key not in _NC_CACHE:
        _NC_CACHE[key] = _build_nc(rows_per_core, c_caps)
    return _NC_CACHE[key]


def _run(pos, trace=False):
    order, c_caps, qneg, cands, cand_idx, row_block = _plan(pos)
    n = pos.shape[0]
    rows_per_core = n // N_CORES
    nc = _get_nc(rows_per_core, c_caps)
    in_maps = []
    for kcore in range(N_CORES):
        m = {"qneg": qneg[kcore]}
        for t in range(len(c_caps)):
            m[f"cand{t}"] = cands[t][kcore]
        in_maps.append(m)
    kw = dict(trace=True, trace_cores=list(range(N_CORES))) if trace else {}
    res = run_bass_kernel_spmd(nc, in_maps, list(range(N_CORES)), **kw)
    vals = np.concatenate([r["vals"] for r in res.results], axis=0)
    slots = np.concatenate([r["idxs"] for r in res.results], axis=0)
    return res, order, cand_idx, row_block, vals, slots


# ----------------------------------------------------------------------------
# public entry point
# ----------------------------------------------------------------------------

def kernel(pos, batch):
    pos = np.asarray(pos, dtype=np.float32)
    batch = np.asarray(batch)
    n = pos.shape[0]
    assert n % (N_CORES * P) == 0 and batch.ndim == 1 and len(batch) == n
    # single-system input (batch constant) is the supported fast path
    assert (batch == batch[0]).all()

    _, order, cand_idx, row_block, vals, slots = _run(pos)
    return _assemble(pos, order, cand_idx, row_block, vals, slots)


def profile_once(np_inputs):
    """Run once with NTFF tracing; return max per-core exec time in ns."""
    _ensure_ntff_hook()
    pos = np.asarray(np_inputs["pos"], dtype=np.float32)
    res, *_ = _run(pos, trace=True)
    print("per-core exec_time_ns:", res.exec_time_ns,
          "mean:", res.mean_exec_time_ns, "max core:", res.max_exec_time_core_id)
    if res.profile_json:
        print("ntff json:", res.profile_json)
    return res.exec_time_ns


def _ensure_ntff_hook():
    """The agent image's antenv lacks axon_hooks; shim it so trace=True works."""
    import sys
    import types
    if "antenv.axon_hooks" not in sys.modules:
        mod = types.ModuleType("antenv.axon_hooks")
        mod._hook = None
        mod.set_axon_ntff_profile_hook = lambda h: setattr(mod, "_hook", h)
        mod.get_axon_ntff_profile_hook = lambda: mod._hook
        sys.modules["antenv.axon_hooks"] = mod
        import antenv
        antenv.axon_hooks = mod
    mod = sys.modules["antenv.axon_hooks"]
    if mod.get_axon_ntff_profile_hook() is None:
        from trn_agent_boot.trn_boot import _ntff_profile_via_ctypes
        mod.set_axon_ntff_profile_hook(
            _ntff_profile_via_ctypes("/opt/axon/libaxon_pjrt.so"))


def _assemble(pos, order, cand_idx, row_block, vals, slots):
    """Host epilogue: slots -> atom ids, cutoff, unsort, edge outputs."""
    n = pos.shape[0]
    d2 = -vals                                       # ascending per row, exact
    dst_s = cand_idx[row_block[:, None], slots.astype(np.int64)]      # [n,K]

    # restore (d2, atom-index) lexicographic order for any exact ties
    ordk = np.lexsort((dst_s, d2), axis=1)
    d2 = np.take_along_axis(d2, ordk, axis=1)
    dst_s = np.take_along_axis(dst_s, ordk, axis=1)

    valid = d2 <= CUTOFF2
    src_orig = order[:, None]                        # original atom id per row
    dst_s = np.where(valid, dst_s, src_orig)         # pad -> self loop

    # un-sort rows back to original atom order
    dst = np.empty((n, K), dtype=np.int64)
    dst[order] = dst_s

    src = np.repeat(np.arange(n, dtype=np.int32), K)
    dst = dst.ravel().astype(np.int32)
    edge_index = np.stack([src, dst]).astype(np.int32)

    edge_vec = pos[src] - pos[dst]                   # raw, unwrapped
    mask = src != dst
    v2 = (edge_vec * edge_vec).sum(axis=-1)
    safe = np.where(mask, v2, np.float32(1.0))
    edge_weight = np.where(mask, np.sqrt(safe), np.float32(0.0)).astype(np.float32)
    return edge_index, edge_weight, edge_vec


# revision 17
# speedup vs baseline: 1.2449x; 1.2449x over previous
"""Periodic radius-graph KNN (minimum-image, K=32) on 8 Trainium2 cores.

Strategy (data-parallel neighbor-list build):
  * Host: sort atoms by spatial cell (8x8x8 grid of 6.25 A cells, box 50 A);
    split the sorted order into blocks of 8 atoms; for each block build a
    candidate list (atoms within cutoff of the block's bounding box, found
    via the cell grid with periodic wrap).  Blocks are then permuted so
    that blocks with similar candidate counts land in the same tile slot
    on every core -- each of the 8 tile slots gets its own compile-time
    candidate capacity C_t (the per-slot max), so DVE/DMA work scales with
    the mean candidate count instead of the global max.
  * Device (SPMD over 8 cores, 1024 atoms each): for every 128-row tile,
    broadcast each 8-row block's candidate coordinate planes across its
    partitions (DMA with partition-broadcast access pattern), compute the
    exact minimum-image squared distance
        d2 = ((mx^2 + my^2) + mz^2),  m_c = min(|dx_c|, 50 - |dx_c|)
    bit-identical to the fp32 reference, and extract the 32 smallest d2
    (with candidate indices) per row via 4 rounds of the DVE max8 /
    max_index / match_replace instructions on the negated keys.
  * Host: map candidate slots back to atom indices, apply the cutoff,
    restore original atom order, and assemble edge_index / edge_weight /
    edge_vec exactly as the reference does.

The fp32 identity min(|dx|, 50-|dx|) reproduces jnp's
`diff - round(diff/box)*box` bit-exactly for box=50 and coords in [0, 50):
the winning branch of the min is always exactly representable, so the
selection and ordering of neighbors match the reference to the last ulp.
"""

from contextlib import ExitStack

import numpy as np

import concourse.bass as bass
import concourse.tile as tile
from concourse import bacc, mybir
from concourse.bass_utils import run_bass_kernel_spmd

N_CORES = 8
K = 32
BOX = 50.0
CELL = 6.25           # 8 cells per dimension
GRID = 8
CUTOFF2 = np.float32(36.0)
BLK = 8               # rows per candidate block
P = 128               # partitions per tile
PAD_COORD = np.float32(1.0e3)   # padding slot coordinate -> d2 ~ 2.7e6 >> 36


# ----------------------------------------------------------------------------
# host-side preprocessing
# ----------------------------------------------------------------------------

def _build_blocks(pos):
    """Sort atoms by cell; per-BLK-row-block candidate index lists."""
    n = pos.shape[0]
    cell = np.minimum((pos // np.float32(CELL)).astype(np.int64), GRID - 1)
    cid = (cell[:, 0] * GRID + cell[:, 1]) * GRID + cell[:, 2]
    order = np.argsort(cid, kind="stable").astype(np.int64)
    pos_s = pos[order]
    cid_s = cid[order]

    atoms_by_cell = [[] for _ in range(GRID ** 3)]
    for i, c in enumerate(cid):
        atoms_by_cell[c].append(i)
    atoms_by_cell = [np.asarray(a, dtype=np.int64) for a in atoms_by_cell]

    n_blocks = n // BLK
    posd = pos.astype(np.float64)
    cand_lists = []
    for blk in range(n_blocks):
        rows = pos_s[blk * BLK:(blk + 1) * BLK].astype(np.float64)
        cells = np.unique(cid_s[blk * BLK:(blk + 1) * BLK])
        seen = set()
        for cc in cells:
            ca, rem = divmod(int(cc), GRID * GRID)
            cb, ccz = divmod(rem, GRID)
            for da in (-1, 0, 1):
                for db in (-1, 0, 1):
                    for dc in (-1, 0, 1):
                        seen.add(
                            (((ca + da) % GRID) * GRID + ((cb + db) % GRID)) * GRID
                            + ((ccz + dc) % GRID))
        seen = [c for c in sorted(seen) if len(atoms_by_cell[c])]
        cand = (np.concatenate([atoms_by_cell[c] for c in seen])
                if seen else np.empty(0, np.int64))
        # exact periodic distance from candidate to block bounding box
        lo = rows.min(axis=0)
        hi = rows.max(axis=0)
        p = posd[cand]
        d = np.zeros(len(cand))
        for k in range(3):
            best = None
            for sh in (-BOX, 0.0, BOX):
                x = p[:, k] + sh
                dd = np.abs(x - np.clip(x, lo[k], hi[k]))
                best = dd if best is None else np.minimum(best, dd)
            d += best * best
        keep = cand[d <= 36.0 + 1e-3]
        cand_lists.append(keep.astype(np.int64))
    return order, cand_lists


def _plan(pos):
    """Full host plan: row permutation, per-slot capacities, DRAM arrays."""
    n = pos.shape[0]
    order0, cand_lists = _build_blocks(pos)
    n_blocks = len(cand_lists)
    ntile = n // (N_CORES * P)
    blk_per_tile = P // BLK
    blocks_per_slot = N_CORES * blk_per_tile

    counts = np.array([len(c) for c in cand_lists])
    rank = np.argsort(counts, kind="stable")       # ascending candidate count
    # Group ranked blocks into ntile slots, then order the slots
    # small, large, small, large, ... so the DMA/ACT pipeline stays ahead of
    # the DVE (a cheap first tile minimises warm-up; alternating sizes avoids
    # a tail stall where the DVE catches up with the prefetch of the most
    # expensive tiles).
    grps = [rank[s * blocks_per_slot:(s + 1) * blocks_per_slot]
            for s in range(ntile)]
    interleave = []
    lo, hi = 0, ntile - 1
    while lo <= hi:
        interleave.append(lo)
        if hi != lo:
            interleave.append(hi)
        lo += 1
        hi -= 1
    grps = [grps[i] for i in interleave]
    c_caps = []
    assign = np.empty((N_CORES, ntile, blk_per_tile), dtype=np.int64)
    for s, grp in enumerate(grps):
        cmax = max(40, int(counts[grp].max()))
        c_caps.append(-(-cmax // 8) * 8)
        assign[:, s, :] = grp.reshape(N_CORES, blk_per_tile)

    # new row order: core-major, then slot, then block, then row-in-block
    block_rows = np.arange(n).reshape(n_blocks, BLK)
    new_rows = block_rows[assign.reshape(-1)].reshape(-1)   # sorted-row indices
    order = order0[new_rows]                                # original atom ids

    pos_s = pos[order]
    # packed per-core qneg: [core][128, 3*ntile]  (col 3t+c = -coord c, tile t)
    qneg = np.empty((N_CORES, P, 3 * ntile), dtype=np.float32)
    for kcore in range(N_CORES):
        slab = -pos_s[kcore * ntile * P:(kcore + 1) * ntile * P]
        for t in range(ntile):
            qneg[kcore, :, 3 * t:3 * t + 3] = slab[t * P:(t + 1) * P]

    # per-slot candidate planes, pre-replicated across each block's rows so the
    # device loads one contiguous [128, 3C] tile per slot (fast DMA, 1 issue
    # per engine) -- list over slots: [N_CORES, P, 3*C_s] f32
    cands = []
    for s in range(ntile):
        C = c_caps[s]
        cx = np.full((N_CORES, blk_per_tile, 3 * C), PAD_COORD, dtype=np.float32)
        for kcore in range(N_CORES):
            for b in range(blk_per_tile):
                cl = cand_lists[assign[kcore, s, b]]
                plane = np.full((3, C), PAD_COORD, dtype=np.float32)
                plane[:, :len(cl)] = pos[cl].T
                cx[kcore, b] = plane.reshape(-1)
        rep = np.broadcast_to(cx[:, :, None, :],
                              (N_CORES, blk_per_tile, BLK, 3 * C))
        cands.append(np.ascontiguousarray(rep).reshape(N_CORES, P, 3 * C))

    cand_idx = np.full((n_blocks, max(c_caps)), -1, dtype=np.int64)
    for b, cl in enumerate(cand_lists):
        cand_idx[b, :len(cl)] = cl
    # block id (in original cell-sorted block numbering) for each new row
    row_block = assign.reshape(-1).repeat(BLK)
    return order, tuple(c_caps), qneg, cands, cand_idx, row_block


# ----------------------------------------------------------------------------
# device kernel (built once per (rows_per_core, c_caps) shape)
# ----------------------------------------------------------------------------

def _build_nc(rows_per_core, c_caps):
    ntile = rows_per_core // P
    blk_per_tile = P // BLK
    assert len(c_caps) == ntile

    nc = bacc.Bacc("TRN2", target_bir_lowering=False, debug=False,
                   enable_asserts=False, num_devices=N_CORES)
    qneg_d = nc.dram_tensor("qneg", [P, 3 * ntile], mybir.dt.float32,
                            kind="ExternalInput").ap()
    cand_d = [nc.dram_tensor(f"cand{t}", [P, 3 * c_caps[t]],
                             mybir.dt.float32, kind="ExternalInput").ap()
              for t in range(ntile)]
    vals_d = nc.dram_tensor("vals", [rows_per_core, K], mybir.dt.float32,
                            kind="ExternalOutput").ap()
    idxs_d = nc.dram_tensor("idxs", [rows_per_core, K], mybir.dt.uint32,
                            kind="ExternalOutput").ap()

    with tile.TileContext(nc) as tc, ExitStack() as ctx:
        pool = ctx.enter_context(tc.tile_pool(name="work", bufs=3))
        opool = ctx.enter_context(tc.tile_pool(name="outs", bufs=ntile))
        cpool = ctx.enter_context(tc.tile_pool(name="consts", bufs=1))
        out_tiles = []

        bias_m50 = cpool.tile([P, 1], mybir.dt.float32)
        nc.gpsimd.memset(bias_m50[:], -50.0)
        bias_0 = cpool.tile([P, 1], mybir.dt.float32)
        nc.gpsimd.memset(bias_0[:], 0.0)
        qneg = cpool.tile([P, 3 * ntile], mybir.dt.float32)
        nc.sync.dma_start(qneg[:], qneg_d[:])

        for t in range(ntile):
            C = c_caps[t]
            xj = pool.tile([P, 3 * C], mybir.dt.float32, tag="xj")
            # 4 issues -> 4 parallel HWDGE engines
            for q in range(4):
                nc.sync.dma_start(xj[32 * q:32 * (q + 1), :],
                                  cand_d[t][32 * q:32 * (q + 1), :])

            # adx = |xj - xi| ; exact fp32 (fma single-rounding == plain sub)
            adx = pool.tile([P, 3 * C], mybir.dt.float32, tag="adx")
            for c in range(3):
                nc.scalar.activation(
                    adx[:, C * c:C * (c + 1)], xj[:, C * c:C * (c + 1)],
                    mybir.ActivationFunctionType.Abs,
                    bias=qneg[:, 3 * t + c:3 * t + c + 1], scale=1.0)

            # msq_c = min(adx^2, (adx-50)^2) == wrapped_diff^2, exactly
            sq1 = pool.tile([P, 3 * C], mybir.dt.float32, tag="xj")  # reuse xj buf
            nc.scalar.activation(sq1[:], adx[:],
                                 mybir.ActivationFunctionType.Square,
                                 bias=bias_0[:])
            sq2 = pool.tile([P, 3 * C], mybir.dt.float32, tag="sq2")
            nc.scalar.activation(sq2[:], adx[:],
                                 mybir.ActivationFunctionType.Square,
                                 bias=bias_m50[:])
            msq = sq2
            nc.vector.tensor_tensor(msq[:], sq1[:], sq2[:], mybir.AluOpType.min)

            # key = -d2 = (-msq_x - msq_y) - msq_z  (negated left-to-right sum)
            key = pool.tile([P, C], mybir.dt.float32, tag="key")
            nc.vector.scalar_tensor_tensor(
                key[:], msq[:, 0:C], -1.0, msq[:, C:2 * C],
                mybir.AluOpType.mult, mybir.AluOpType.subtract)
            nc.vector.tensor_tensor(key[:], key[:], msq[:, 2 * C:3 * C],
                                    mybir.AluOpType.subtract)

            # 4 rounds of 8-way extraction = top-32 ascending d2
            out = opool.tile([P, 2 * K], mybir.dt.uint32, tag="out")
            out_tiles.append(out)
            vals = out[:, 0:K].bitcast(mybir.dt.float32)
            idxs = out[:, K:2 * K]
            for r in range(K // 8):
                v8 = vals[:, 8 * r:8 * (r + 1)]
                nc.vector.max(v8, key[:])
                nc.vector.max_index(idxs[:, 8 * r:8 * (r + 1)], v8, key[:])
                if r != K // 8 - 1:
                    nc.vector.match_replace(key[:], v8, key[:], -3.0e38)

        # all output DMAs at the end on the otherwise-idle gpsimd queue, so no
        # engine's instruction stream ever blocks a later tile's work on them
        for t, out in enumerate(out_tiles):
            nc.gpsimd.dma_start(vals_d[t * P:(t + 1) * P, :],
                                out[:, 0:K].bitcast(mybir.dt.float32))
            nc.gpsimd.dma_start(idxs_d[t * P:(t + 1) * P, :], out[:, K:2 * K])
    nc.compile()
    return nc


_NC_CACHE = {}


def _get_nc(rows_per_core, c_caps):
    key = (rows_per_core, c_caps)
    if key not in _NC_CACHE:
        _NC_CACHE[key] = _build_nc(rows_per_core, c_caps)
    return _NC_CACHE[key]


def _run(pos, trace=False):
    order, c_caps, qneg, cands, cand_idx, row_block = _plan(pos)
    n = pos.shape[0]
    rows_per_core = n // N_CORES
    nc = _get_nc(rows_per_core, c_caps)
    in_maps = []
    for kcore in range(N_CORES):
        m = {"qneg": qneg[kcore]}
        for t in range(len(c_caps)):
            m[f"cand{t}"] = cands[t][kcore]
        in_maps.append(m)
    kw = dict(trace=True, trace_cores=list(range(N_CORES))) if trace else {}
    res = run_bass_kernel_spmd(nc, in_maps, list(range(N_CORES)), **kw)
    vals = np.concatenate([r["vals"] for r in res.results], axis=0)
    slots = np.concatenate([r["idxs"] for r in res.results], axis=0)
    return res, order, cand_idx, row_block, vals, slots


# ----------------------------------------------------------------------------
# public entry point
# ----------------------------------------------------------------------------

def kernel(pos, batch):
    pos = np.asarray(pos, dtype=np.float32)
    batch = np.asarray(batch)
    n = pos.shape[0]
    assert n % (N_CORES * P) == 0 and batch.ndim == 1 and len(batch) == n
    # single-system input (batch constant) is the supported fast path
    assert (batch == batch[0]).all()

    _, order, cand_idx, row_block, vals, slots = _run(pos)
    return _assemble(pos, order, cand_idx, row_block, vals, slots)


def profile_once(np_inputs):
    """Run once with NTFF tracing; return max per-core exec time in ns."""
    _ensure_ntff_hook()
    pos = np.asarray(np_inputs["pos"], dtype=np.float32)
    res, *_ = _run(pos, trace=True)
    print("per-core exec_time_ns:", res.exec_time_ns,
          "mean:", res.mean_exec_time_ns, "max core:", res.max_exec_time_core_id)
    if res.profile_json:
        print("ntff json:", res.profile_json)
    return res.exec_time_ns


def _ensure_ntff_hook():
    """The agent image's antenv lacks axon_hooks; shim it so trace=True works."""
    import sys
    import types
    if "antenv.axon_hooks" not in sys.modules:
        mod = types.ModuleType("antenv.axon_hooks")
        mod._hook = None
        mod.set_axon_ntff_profile_hook = lambda h: setattr(mod, "_hook", h)
        mod.get_axon_ntff_profile_hook = lambda: mod._hook
        sys.modules["antenv.axon_hooks"] = mod
        import antenv
        antenv.axon_hooks = mod
    mod = sys.modules["antenv.axon_hooks"]
    if mod.get_axon_ntff_profile_hook() is None:
        from trn_agent_boot.trn_boot import _ntff_profile_via_ctypes
        mod.set_axon_ntff_profile_hook(
            _ntff_profile_via_ctypes("/opt/axon/libaxon_pjrt.so"))


def _assemble(pos, order, cand_idx, row_block, vals, slots):
    """Host epilogue: slots -> atom ids, cutoff, unsort, edge outputs."""
    n = pos.shape[0]
    d2 = -vals                                       # ascending per row, exact
    dst_s = cand_idx[row_block[:, None], slots.astype(np.int64)]      # [n,K]

    # restore (d2, atom-index) lexicographic order for any exact ties
    ordk = np.lexsort((dst_s, d2), axis=1)
    d2 = np.take_along_axis(d2, ordk, axis=1)
    dst_s = np.take_along_axis(dst_s, ordk, axis=1)

    valid = d2 <= CUTOFF2
    src_orig = order[:, None]                        # original atom id per row
    dst_s = np.where(valid, dst_s, src_orig)         # pad -> self loop

    # un-sort rows back to original atom order
    dst = np.empty((n, K), dtype=np.int64)
    dst[order] = dst_s

    src = np.repeat(np.arange(n, dtype=np.int32), K)
    dst = dst.ravel().astype(np.int32)
    edge_index = np.stack([src, dst]).astype(np.int32)

    edge_vec = pos[src] - pos[dst]                   # raw, unwrapped
    mask = src != dst
    v2 = (edge_vec * edge_vec).sum(axis=-1)
    safe = np.where(mask, v2, np.float32(1.0))
    edge_weight = np.where(mask, np.sqrt(safe), np.float32(0.0)).astype(np.float32)
    return edge_index, edge_weight, edge_vec


# revision 20
# speedup vs baseline: 1.3663x; 1.0975x over previous
"""Periodic radius-graph KNN (minimum-image, K=32) on 8 Trainium2 cores.

Strategy (data-parallel neighbor-list build):
  * Host: sort atoms by spatial cell (8x8x8 grid of 6.25 A cells, box 50 A);
    split the sorted order into blocks of 8 atoms; for each block build a
    candidate list (atoms within cutoff of the block's bounding box, found
    via the cell grid with periodic wrap).  Blocks are then permuted so
    that blocks with similar candidate counts land in the same tile slot
    on every core -- each of the 8 tile slots gets its own compile-time
    candidate capacity C_t (the per-slot max), so DVE/DMA work scales with
    the mean candidate count instead of the global max.
  * Device (SPMD over 8 cores, 1024 atoms each): for every 128-row tile,
    broadcast each 8-row block's candidate coordinate planes across its
    partitions (DMA with partition-broadcast access pattern), compute the
    exact minimum-image squared distance
        d2 = ((mx^2 + my^2) + mz^2),  m_c = min(|dx_c|, 50 - |dx_c|)
    bit-identical to the fp32 reference, and extract the 32 smallest d2
    (with candidate indices) per row via 4 rounds of the DVE max8 /
    max_index / match_replace instructions on the negated keys.
  * Host: map candidate slots back to atom indices, apply the cutoff,
    restore original atom order, and assemble edge_index / edge_weight /
    edge_vec exactly as the reference does.

The fp32 identity min(|dx|, 50-|dx|) reproduces jnp's
`diff - round(diff/box)*box` bit-exactly for box=50 and coords in [0, 50):
the winning branch of the min is always exactly representable, so the
selection and ordering of neighbors match the reference to the last ulp.
"""

from contextlib import ExitStack

import numpy as np

import concourse.bass as bass
import concourse.tile as tile
from concourse import bacc, mybir
from concourse.bass_utils import run_bass_kernel_spmd

N_CORES = 8
K = 32
BOX = 50.0
CELL = 6.25           # 8 cells per dimension
GRID = 8
CUTOFF2 = np.float32(36.0)
BLK = 4               # rows per candidate block
P = 128               # partitions per tile
PAD_COORD = np.float32(1.0e3)   # padding slot coordinate -> d2 ~ 2.7e6 >> 36


# ----------------------------------------------------------------------------
# host-side preprocessing
# ----------------------------------------------------------------------------

def _build_blocks(pos):
    """Sort atoms by cell; per-BLK-row-block candidate index lists."""
    n = pos.shape[0]
    cell = np.minimum((pos // np.float32(CELL)).astype(np.int64), GRID - 1)
    cid = (cell[:, 0] * GRID + cell[:, 1]) * GRID + cell[:, 2]
    order = np.argsort(cid, kind="stable").astype(np.int64)
    pos_s = pos[order]
    cid_s = cid[order]

    atoms_by_cell = [[] for _ in range(GRID ** 3)]
    for i, c in enumerate(cid):
        atoms_by_cell[c].append(i)
    atoms_by_cell = [np.asarray(a, dtype=np.int64) for a in atoms_by_cell]

    n_blocks = n // BLK
    posd = pos.astype(np.float64)
    cand_lists = []
    for blk in range(n_blocks):
        rows = pos_s[blk * BLK:(blk + 1) * BLK].astype(np.float64)
        cells = np.unique(cid_s[blk * BLK:(blk + 1) * BLK])
        seen = set()
        for cc in cells:
            ca, rem = divmod(int(cc), GRID * GRID)
            cb, ccz = divmod(rem, GRID)
            for da in (-1, 0, 1):
                for db in (-1, 0, 1):
                    for dc in (-1, 0, 1):
                        seen.add(
                            (((ca + da) % GRID) * GRID + ((cb + db) % GRID)) * GRID
                            + ((ccz + dc) % GRID))
        seen = [c for c in sorted(seen) if len(atoms_by_cell[c])]
        cand = (np.concatenate([atoms_by_cell[c] for c in seen])
                if seen else np.empty(0, np.int64))
        # exact periodic distance from candidate to block bounding box
        lo = rows.min(axis=0)
        hi = rows.max(axis=0)
        p = posd[cand]
        d = np.zeros(len(cand))
        for k in range(3):
            best = None
            for sh in (-BOX, 0.0, BOX):
                x = p[:, k] + sh
                dd = np.abs(x - np.clip(x, lo[k], hi[k]))
                best = dd if best is None else np.minimum(best, dd)
            d += best * best
        keep = cand[d <= 36.0 + 1e-3]
        cand_lists.append(keep.astype(np.int64))
    return order, cand_lists


def _plan(pos):
    """Full host plan: row permutation, per-slot capacities, DRAM arrays."""
    n = pos.shape[0]
    order0, cand_lists = _build_blocks(pos)
    n_blocks = len(cand_lists)
    ntile = n // (N_CORES * P)
    blk_per_tile = P // BLK
    blocks_per_slot = N_CORES * blk_per_tile

    counts = np.array([len(c) for c in cand_lists])
    rank = np.argsort(counts, kind="stable")       # ascending candidate count
    # Group ranked blocks into ntile slots, then order the slots
    # small, large, small, large, ... so the DMA/ACT pipeline stays ahead of
    # the DVE (a cheap first tile minimises warm-up; alternating sizes avoids
    # a tail stall where the DVE catches up with the prefetch of the most
    # expensive tiles).
    grps = [rank[s * blocks_per_slot:(s + 1) * blocks_per_slot]
            for s in range(ntile)]
    interleave = []
    lo, hi = 0, ntile - 1
    while lo <= hi:
        interleave.append(lo)
        if hi != lo:
            interleave.append(hi)
        lo += 1
        hi -= 1
    grps = [grps[i] for i in interleave]
    c_caps = []
    assign = np.empty((N_CORES, ntile, blk_per_tile), dtype=np.int64)
    for s, grp in enumerate(grps):
        cmax = max(40, int(counts[grp].max()))
        c_caps.append(-(-cmax // 8) * 8)
        assign[:, s, :] = grp.reshape(N_CORES, blk_per_tile)

    # new row order: core-major, then slot, then block, then row-in-block
    block_rows = np.arange(n).reshape(n_blocks, BLK)
    new_rows = block_rows[assign.reshape(-1)].reshape(-1)   # sorted-row indices
    order = order0[new_rows]                                # original atom ids

    pos_s = pos[order]
    # packed per-core qneg: [core][128, 3*ntile]  (col 3t+c = -coord c, tile t)
    qneg = np.empty((N_CORES, P, 3 * ntile), dtype=np.float32)
    for kcore in range(N_CORES):
        slab = -pos_s[kcore * ntile * P:(kcore + 1) * ntile * P]
        for t in range(ntile):
            qneg[kcore, :, 3 * t:3 * t + 3] = slab[t * P:(t + 1) * P]

    # per-slot candidate planes, pre-replicated across each block's rows so the
    # device loads one contiguous [128, 3C] tile per slot (fast DMA, 1 issue
    # per engine) -- list over slots: [N_CORES, P, 3*C_s] f32
    cands = []
    for s in range(ntile):
        C = c_caps[s]
        cx = np.full((N_CORES, blk_per_tile, 3 * C), PAD_COORD, dtype=np.float32)
        for kcore in range(N_CORES):
            for b in range(blk_per_tile):
                cl = cand_lists[assign[kcore, s, b]]
                plane = np.full((3, C), PAD_COORD, dtype=np.float32)
                plane[:, :len(cl)] = pos[cl].T
                cx[kcore, b] = plane.reshape(-1)
        rep = np.broadcast_to(cx[:, :, None, :],
                              (N_CORES, blk_per_tile, BLK, 3 * C))
        cands.append(np.ascontiguousarray(rep).reshape(N_CORES, P, 3 * C))

    cand_idx = np.full((n_blocks, max(c_caps)), -1, dtype=np.int64)
    for b, cl in enumerate(cand_lists):
        cand_idx[b, :len(cl)] = cl
    # block id (in original cell-sorted block numbering) for each new row
    row_block = assign.reshape(-1).repeat(BLK)
    return order, tuple(c_caps), qneg, cands, cand_idx, row_block


# ----------------------------------------------------------------------------
# device kernel (built once per (rows_per_core, c_caps) shape)
# ----------------------------------------------------------------------------

def _build_nc(rows_per_core, c_caps):
    ntile = rows_per_core // P
    blk_per_tile = P // BLK
    assert len(c_caps) == ntile

    nc = bacc.Bacc("TRN2", target_bir_lowering=False, debug=False,
                   enable_asserts=False, num_devices=N_CORES)
    qneg_d = nc.dram_tensor("qneg", [P, 3 * ntile], mybir.dt.float32,
                            kind="ExternalInput").ap()
    cand_d = [nc.dram_tensor(f"cand{t}", [P, 3 * c_caps[t]],
                             mybir.dt.float32, kind="ExternalInput").ap()
              for t in range(ntile)]
    vals_d = nc.dram_tensor("vals", [rows_per_core, K], mybir.dt.float32,
                            kind="ExternalOutput").ap()
    idxs_d = nc.dram_tensor("idxs", [rows_per_core, K], mybir.dt.uint32,
                            kind="ExternalOutput").ap()

    with tile.TileContext(nc) as tc, ExitStack() as ctx:
        pool = ctx.enter_context(tc.tile_pool(name="work", bufs=3))
        opool = ctx.enter_context(tc.tile_pool(name="outs", bufs=ntile))
        cpool = ctx.enter_context(tc.tile_pool(name="consts", bufs=1))
        out_tiles = []

        bias_m50 = cpool.tile([P, 1], mybir.dt.float32)
        nc.gpsimd.memset(bias_m50[:], -50.0)
        bias_0 = cpool.tile([P, 1], mybir.dt.float32)
        nc.gpsimd.memset(bias_0[:], 0.0)
        qneg = cpool.tile([P, 3 * ntile], mybir.dt.float32)
        nc.sync.dma_start(qneg[:], qneg_d[:])

        for t in range(ntile):
            C = c_caps[t]
            xj = pool.tile([P, 3 * C], mybir.dt.float32, tag="xj")
            # 4 issues -> 4 parallel HWDGE engines
            for q in range(4):
                nc.sync.dma_start(xj[32 * q:32 * (q + 1), :],
                                  cand_d[t][32 * q:32 * (q + 1), :])

            # adx = |xj - xi| ; exact fp32 (fma single-rounding == plain sub)
            adx = pool.tile([P, 3 * C], mybir.dt.float32, tag="adx")
            for c in range(3):
                nc.scalar.activation(
                    adx[:, C * c:C * (c + 1)], xj[:, C * c:C * (c + 1)],
                    mybir.ActivationFunctionType.Abs,
                    bias=qneg[:, 3 * t + c:3 * t + c + 1], scale=1.0)

            # msq_c = min(adx^2, (adx-50)^2) == wrapped_diff^2, exactly
            sq1 = pool.tile([P, 3 * C], mybir.dt.float32, tag="xj")  # reuse xj buf
            nc.scalar.activation(sq1[:], adx[:],
                                 mybir.ActivationFunctionType.Square,
                                 bias=bias_0[:])
            sq2 = pool.tile([P, 3 * C], mybir.dt.float32, tag="sq2")
            nc.scalar.activation(sq2[:], adx[:],
                                 mybir.ActivationFunctionType.Square,
                                 bias=bias_m50[:])
            msq = sq2
            nc.vector.tensor_tensor(msq[:], sq1[:], sq2[:], mybir.AluOpType.min)

            # key = -d2 = (-msq_x - msq_y) - msq_z  (negated left-to-right sum)
            key = pool.tile([P, C], mybir.dt.float32, tag="key")
            nc.vector.scalar_tensor_tensor(
                key[:], msq[:, 0:C], -1.0, msq[:, C:2 * C],
                mybir.AluOpType.mult, mybir.AluOpType.subtract)
            nc.vector.tensor_tensor(key[:], key[:], msq[:, 2 * C:3 * C],
                                    mybir.AluOpType.subtract)

            # 4 rounds of 8-way extraction = top-32 ascending d2
            out = opool.tile([P, 2 * K], mybir.dt.uint32, tag="out")
            out_tiles.append(out)
            vals = out[:, 0:K].bitcast(mybir.dt.float32)
            idxs = out[:, K:2 * K]
            for r in range(K // 8):
                v8 = vals[:, 8 * r:8 * (r + 1)]
                nc.vector.max(v8, key[:])
                nc.vector.max_index(idxs[:, 8 * r:8 * (r + 1)], v8, key[:])
                if r != K // 8 - 1:
                    nc.vector.match_replace(key[:], v8, key[:], -3.0e38)

        # all output DMAs at the end on the otherwise-idle gpsimd queue, so no
        # engine's instruction stream ever blocks a later tile's work on them
        for t, out in enumerate(out_tiles):
            nc.gpsimd.dma_start(vals_d[t * P:(t + 1) * P, :],
                                out[:, 0:K].bitcast(mybir.dt.float32))
            nc.gpsimd.dma_start(idxs_d[t * P:(t + 1) * P, :], out[:, K:2 * K])
    nc.compile()
    return nc


_NC_CACHE = {}


def _get_nc(rows_per_core, c_caps):
    key = (rows_per_core, c_caps)
    if key not in _NC_CACHE:
        _NC_CACHE[key] = _build_nc(rows_per_core, c_caps)
    return _NC_CACHE[key]


def _run(pos, trace=False):
    order, c_caps, qneg, cands, cand_idx, row_block = _plan(pos)
    n = pos.shape[0]
    rows_per_core = n // N_CORES
    nc = _get_nc(rows_per_core, c_caps)
    in_maps = []
    for kcore in range(N_CORES):
        m = {"qneg": qneg[kcore]}
        for t in range(len(c_caps)):
            m[f"cand{t}"] = cands[t][kcore]
        in_maps.append(m)
    kw = dict(trace=True, trace_cores=list(range(N_CORES))) if trace else {}
    res = run_bass_kernel_spmd(nc, in_maps, list(range(N_CORES)), **kw)
    vals = np.concatenate([r["vals"] for r in res.results], axis=0)
    slots = np.concatenate([r["idxs"] for r in res.results], axis=0)
    return res, order, cand_idx, row_block, vals, slots


# ----------------------------------------------------------------------------
# public entry point
# ----------------------------------------------------------------------------

def kernel(pos, batch):
    pos = np.asarray(pos, dtype=np.float32)
    batch = np.asarray(batch)
    n = pos.shape[0]
    assert n % (N_CORES * P) == 0 and batch.ndim == 1 and len(batch) == n
    # single-system input (batch constant) is the supported fast path
    assert (batch == batch[0]).all()

    _, order, cand_idx, row_block, vals, slots = _run(pos)
    return _assemble(pos, order, cand_idx, row_block, vals, slots)


def profile_once(np_inputs):
    """Run once with NTFF tracing; return max per-core exec time in ns."""
    _ensure_ntff_hook()
    pos = np.asarray(np_inputs["pos"], dtype=np.float32)
    res, *_ = _run(pos, trace=True)
    print("per-core exec_time_ns:", res.exec_time_ns,
          "mean:", res.mean_exec_time_ns, "max core:", res.max_exec_time_core_id)
    if res.profile_json:
        print("ntff json:", res.profile_json)
    return res.exec_time_ns


def _ensure_ntff_hook():
    """The agent image's antenv lacks axon_hooks; shim it so trace=True works."""
    import sys
    import types
    if "antenv.axon_hooks" not in sys.modules:
        mod = types.ModuleType("antenv.axon_hooks")
        mod._hook = None
        mod.set_axon_ntff_profile_hook = lambda h: setattr(mod, "_hook", h)
        mod.get_axon_ntff_profile_hook = lambda: mod._hook
        sys.modules["antenv.axon_hooks"] = mod
        import antenv
        antenv.axon_hooks = mod
    mod = sys.modules["antenv.axon_hooks"]
    if mod.get_axon_ntff_profile_hook() is None:
        from trn_agent_boot.trn_boot import _ntff_profile_via_ctypes
        mod.set_axon_ntff_profile_hook(
            _ntff_profile_via_ctypes("/opt/axon/libaxon_pjrt.so"))


def _assemble(pos, order, cand_idx, row_block, vals, slots):
    """Host epilogue: slots -> atom ids, cutoff, unsort, edge outputs."""
    n = pos.shape[0]
    d2 = -vals                                       # ascending per row, exact
    dst_s = cand_idx[row_block[:, None], slots.astype(np.int64)]      # [n,K]

    # restore (d2, atom-index) lexicographic order for any exact ties
    ordk = np.lexsort((dst_s, d2), axis=1)
    d2 = np.take_along_axis(d2, ordk, axis=1)
    dst_s = np.take_along_axis(dst_s, ordk, axis=1)

    valid = d2 <= CUTOFF2
    src_orig = order[:, None]                        # original atom id per row
    dst_s = np.where(valid, dst_s, src_orig)         # pad -> self loop

    # un-sort rows back to original atom order
    dst = np.empty((n, K), dtype=np.int64)
    dst[order] = dst_s

    src = np.repeat(np.arange(n, dtype=np.int32), K)
    dst = dst.ravel().astype(np.int32)
    edge_index = np.stack([src, dst]).astype(np.int32)

    edge_vec = pos[src] - pos[dst]                   # raw, unwrapped
    mask = src != dst
    v2 = (edge_vec * edge_vec).sum(axis=-1)
    safe = np.where(mask, v2, np.float32(1.0))
    edge_weight = np.where(mask, np.sqrt(safe), np.float32(0.0)).astype(np.float32)
    return edge_index, edge_weight, edge_vec


# revision 22
# speedup vs baseline: 1.3756x; 1.0068x over previous
"""Periodic radius-graph KNN (minimum-image, K=32) on 8 Trainium2 cores.

Strategy (data-parallel neighbor-list build):
  * Host: sort atoms by spatial cell (8x8x8 grid of 6.25 A cells, box 50 A);
    split the sorted order into blocks of 8 atoms; for each block build a
    candidate list (atoms within cutoff of the block's bounding box, found
    via the cell grid with periodic wrap).  Blocks are then permuted so
    that blocks with similar candidate counts land in the same tile slot
    on every core -- each of the 8 tile slots gets its own compile-time
    candidate capacity C_t (the per-slot max), so DVE/DMA work scales with
    the mean candidate count instead of the global max.
  * Device (SPMD over 8 cores, 1024 atoms each): for every 128-row tile,
    broadcast each 8-row block's candidate coordinate planes across its
    partitions (DMA with partition-broadcast access pattern), compute the
    exact minimum-image squared distance
        d2 = ((mx^2 + my^2) + mz^2),  m_c = min(|dx_c|, 50 - |dx_c|)
    bit-identical to the fp32 reference, and extract the 32 smallest d2
    (with candidate indices) per row via 4 rounds of the DVE max8 /
    max_index / match_replace instructions on the negated keys.
  * Host: map candidate slots back to atom indices, apply the cutoff,
    restore original atom order, and assemble edge_index / edge_weight /
    edge_vec exactly as the reference does.

The fp32 identity min(|dx|, 50-|dx|) reproduces jnp's
`diff - round(diff/box)*box` bit-exactly for box=50 and coords in [0, 50):
the winning branch of the min is always exactly representable, so the
selection and ordering of neighbors match the reference to the last ulp.
"""

from contextlib import ExitStack

import numpy as np

import concourse.bass as bass
import concourse.tile as tile
from concourse import bacc, mybir
from concourse.bass_utils import run_bass_kernel_spmd

N_CORES = 8
K = 32
BOX = 50.0
CELL = 6.25           # 8 cells per dimension
GRID = 8
CUTOFF2 = np.float32(36.0)
BLK = 4               # rows per candidate block
P = 128               # partitions per tile
PAD_COORD = np.float32(1.0e3)   # padding slot coordinate -> d2 ~ 2.7e6 >> 36


# ----------------------------------------------------------------------------
# host-side preprocessing
# ----------------------------------------------------------------------------

def _build_blocks(pos):
    """Sort atoms by cell; per-BLK-row-block candidate index lists."""
    n = pos.shape[0]
    cell = np.minimum((pos // np.float32(CELL)).astype(np.int64), GRID - 1)
    cid = (cell[:, 0] * GRID + cell[:, 1]) * GRID + cell[:, 2]
    order = np.argsort(cid, kind="stable").astype(np.int64)
    pos_s = pos[order]
    cid_s = cid[order]

    atoms_by_cell = [[] for _ in range(GRID ** 3)]
    for i, c in enumerate(cid):
        atoms_by_cell[c].append(i)
    atoms_by_cell = [np.asarray(a, dtype=np.int64) for a in atoms_by_cell]

    n_blocks = n // BLK
    posd = pos.astype(np.float64)
    cand_lists = []
    for blk in range(n_blocks):
        rows = pos_s[blk * BLK:(blk + 1) * BLK].astype(np.float64)
        cells = np.unique(cid_s[blk * BLK:(blk + 1) * BLK])
        seen = set()
        for cc in cells:
            ca, rem = divmod(int(cc), GRID * GRID)
            cb, ccz = divmod(rem, GRID)
            for da in (-1, 0, 1):
                for db in (-1, 0, 1):
                    for dc in (-1, 0, 1):
                        seen.add(
                            (((ca + da) % GRID) * GRID + ((cb + db) % GRID)) * GRID
                            + ((ccz + dc) % GRID))
        seen = [c for c in sorted(seen) if len(atoms_by_cell[c])]
        cand = (np.concatenate([atoms_by_cell[c] for c in seen])
                if seen else np.empty(0, np.int64))
        # exact periodic distance from candidate to block bounding box
        lo = rows.min(axis=0)
        hi = rows.max(axis=0)
        p = posd[cand]
        d = np.zeros(len(cand))
        for k in range(3):
            best = None
            for sh in (-BOX, 0.0, BOX):
                x = p[:, k] + sh
                dd = np.abs(x - np.clip(x, lo[k], hi[k]))
                best = dd if best is None else np.minimum(best, dd)
            d += best * best
        keep = cand[d <= 36.0 + 1e-3]
        cand_lists.append(keep.astype(np.int64))
    return order, cand_lists


def _plan(pos):
    """Full host plan: row permutation, per-slot capacities, DRAM arrays."""
    n = pos.shape[0]
    order0, cand_lists = _build_blocks(pos)
    n_blocks = len(cand_lists)
    ntile = n // (N_CORES * P)
    blk_per_tile = P // BLK
    blocks_per_slot = N_CORES * blk_per_tile

    counts = np.array([len(c) for c in cand_lists])
    rank = np.argsort(counts, kind="stable")       # ascending candidate count
    # Group ranked blocks into ntile slots, then order the slots
    # small, large, small, large, ... so the DMA/ACT pipeline stays ahead of
    # the DVE (a cheap first tile minimises warm-up; alternating sizes avoids
    # a tail stall where the DVE catches up with the prefetch of the most
    # expensive tiles).
    grps = [rank[s * blocks_per_slot:(s + 1) * blocks_per_slot]
            for s in range(ntile)]
    interleave = []
    lo, hi = 0, ntile - 1
    while lo <= hi:
        interleave.append(lo)
        if hi != lo:
            interleave.append(hi)
        lo += 1
        hi -= 1
    grps = [grps[i] for i in interleave]
    c_caps = []
    assign = np.empty((N_CORES, ntile, blk_per_tile), dtype=np.int64)
    for s, grp in enumerate(grps):
        cmax = max(40, int(counts[grp].max()))
        c_caps.append(-(-cmax // 8) * 8)
        assign[:, s, :] = grp.reshape(N_CORES, blk_per_tile)

    # new row order: core-major, then slot, then block, then row-in-block
    block_rows = np.arange(n).reshape(n_blocks, BLK)
    new_rows = block_rows[assign.reshape(-1)].reshape(-1)   # sorted-row indices
    order = order0[new_rows]                                # original atom ids

    pos_s = pos[order]
    # packed per-core qneg: [core][128, 3*ntile]  (col 3t+c = -coord c, tile t)
    qneg = np.empty((N_CORES, P, 3 * ntile), dtype=np.float32)
    for kcore in range(N_CORES):
        slab = -pos_s[kcore * ntile * P:(kcore + 1) * ntile * P]
        for t in range(ntile):
            qneg[kcore, :, 3 * t:3 * t + 3] = slab[t * P:(t + 1) * P]

    # per-slot candidate planes, pre-replicated across each block's rows so the
    # device loads one contiguous [128, 3C] tile per slot (fast DMA, 1 issue
    # per engine) -- list over slots: [N_CORES, P, 3*C_s] f32
    cands = []
    for s in range(ntile):
        C = c_caps[s]
        cx = np.full((N_CORES, blk_per_tile, 3 * C), PAD_COORD, dtype=np.float32)
        for kcore in range(N_CORES):
            for b in range(blk_per_tile):
                cl = cand_lists[assign[kcore, s, b]]
                plane = np.full((3, C), PAD_COORD, dtype=np.float32)
                plane[:, :len(cl)] = pos[cl].T
                cx[kcore, b] = plane.reshape(-1)
        rep = np.broadcast_to(cx[:, :, None, :],
                              (N_CORES, blk_per_tile, BLK, 3 * C))
        cands.append(np.ascontiguousarray(rep).reshape(N_CORES, P, 3 * C))

    cand_idx = np.full((n_blocks, max(c_caps)), -1, dtype=np.int64)
    for b, cl in enumerate(cand_lists):
        cand_idx[b, :len(cl)] = cl
    # block id (in original cell-sorted block numbering) for each new row
    row_block = assign.reshape(-1).repeat(BLK)
    return order, tuple(c_caps), qneg, cands, cand_idx, row_block


# ----------------------------------------------------------------------------
# device kernel (built once per (rows_per_core, c_caps) shape)
# ----------------------------------------------------------------------------

def _build_nc(rows_per_core, c_caps):
    ntile = rows_per_core // P
    blk_per_tile = P // BLK
    assert len(c_caps) == ntile

    nc = bacc.Bacc("TRN2", target_bir_lowering=False, debug=False,
                   enable_asserts=False, num_devices=N_CORES)
    qneg_d = nc.dram_tensor("qneg", [P, 3 * ntile], mybir.dt.float32,
                            kind="ExternalInput").ap()
    cand_d = [nc.dram_tensor(f"cand{t}", [P, 3 * c_caps[t]],
                             mybir.dt.float32, kind="ExternalInput").ap()
              for t in range(ntile)]
    vals_d = nc.dram_tensor("vals", [rows_per_core, K], mybir.dt.float32,
                            kind="ExternalOutput").ap()
    idxs_d = nc.dram_tensor("idxs", [rows_per_core, K], mybir.dt.uint32,
                            kind="ExternalOutput").ap()

    with tile.TileContext(nc) as tc, ExitStack() as ctx:
        pool = ctx.enter_context(tc.tile_pool(name="work", bufs=3))
        opool = ctx.enter_context(tc.tile_pool(name="outs", bufs=ntile))
        cpool = ctx.enter_context(tc.tile_pool(name="consts", bufs=1))
        out_tiles = []

        bias_m50 = cpool.tile([P, 1], mybir.dt.float32)
        nc.gpsimd.memset(bias_m50[:], -50.0)
        bias_0 = cpool.tile([P, 1], mybir.dt.float32)
        nc.gpsimd.memset(bias_0[:], 0.0)
        qneg = cpool.tile([P, 3 * ntile], mybir.dt.float32)
        nc.sync.dma_start(qneg[:], qneg_d[:])
        # warm the ACT table set while the first candidate DMA is in flight
        warm = cpool.tile([P, 8], mybir.dt.float32)
        nc.gpsimd.memset(warm[:], 1.0)
        nc.scalar.activation(warm[:], warm[:], mybir.ActivationFunctionType.Abs,
                             bias=bias_0[:], scale=1.0)
        nc.scalar.activation(warm[:], warm[:],
                             mybir.ActivationFunctionType.Square, bias=bias_0[:])

        for t in range(ntile):
            C = c_caps[t]
            xj = pool.tile([P, 3 * C], mybir.dt.float32, tag="xj")
            # 4 issues -> 4 parallel HWDGE engines
            for q in range(4):
                nc.sync.dma_start(xj[32 * q:32 * (q + 1), :],
                                  cand_d[t][32 * q:32 * (q + 1), :])

            # adx = |xj - xi| ; exact fp32 (fma single-rounding == plain sub)
            adx = pool.tile([P, 3 * C], mybir.dt.float32, tag="adx")
            for c in range(3):
                nc.scalar.activation(
                    adx[:, C * c:C * (c + 1)], xj[:, C * c:C * (c + 1)],
                    mybir.ActivationFunctionType.Abs,
                    bias=qneg[:, 3 * t + c:3 * t + c + 1], scale=1.0)

            # msq_c = min(adx^2, (adx-50)^2) == wrapped_diff^2, exactly
            sq1 = pool.tile([P, 3 * C], mybir.dt.float32, tag="xj")  # reuse xj buf
            nc.scalar.activation(sq1[:], adx[:],
                                 mybir.ActivationFunctionType.Square,
                                 bias=bias_0[:])
            sq2 = pool.tile([P, 3 * C], mybir.dt.float32, tag="sq2")
            nc.scalar.activation(sq2[:], adx[:],
                                 mybir.ActivationFunctionType.Square,
                                 bias=bias_m50[:])
            msq = sq2
            nc.vector.tensor_tensor(msq[:], sq1[:], sq2[:], mybir.AluOpType.min)

            # key = -d2 = (-msq_x - msq_y) - msq_z  (negated left-to-right sum)
            key = pool.tile([P, C], mybir.dt.float32, tag="key")
            nc.vector.scalar_tensor_tensor(
                key[:], msq[:, 0:C], -1.0, msq[:, C:2 * C],
                mybir.AluOpType.mult, mybir.AluOpType.subtract)
            nc.vector.tensor_tensor(key[:], key[:], msq[:, 2 * C:3 * C],
                                    mybir.AluOpType.subtract)

            # 4 rounds of 8-way extraction = top-32 ascending d2
            out = opool.tile([P, 2 * K], mybir.dt.uint32, tag="out")
            vals = out[:, 0:K].bitcast(mybir.dt.float32)
            idxs = out[:, K:2 * K]
            for r in range(K // 8):
                v8 = vals[:, 8 * r:8 * (r + 1)]
                nc.vector.max(v8, key[:])
                nc.vector.max_index(idxs[:, 8 * r:8 * (r + 1)], v8, key[:])
                if r != K // 8 - 1:
                    nc.vector.match_replace(key[:], v8, key[:], -3.0e38)

            # outputs on the gpsimd queue (which carries nothing else per-tile,
            # so no later tile's input path ever blocks on them)
            nc.gpsimd.dma_start(vals_d[t * P:(t + 1) * P, :], vals[:])
            nc.gpsimd.dma_start(idxs_d[t * P:(t + 1) * P, :], idxs[:])
    nc.compile()
    return nc


_NC_CACHE = {}


def _get_nc(rows_per_core, c_caps):
    key = (rows_per_core, c_caps)
    if key not in _NC_CACHE:
        _NC_CACHE[key] = _build_nc(rows_per_core, c_caps)
    return _NC_CACHE[key]


def _run(pos, trace=False):
    order, c_caps, qneg, cands, cand_idx, row_block = _plan(pos)
    n = pos.shape[0]
    rows_per_core = n // N_CORES
    nc = _get_nc(rows_per_core, c_caps)
    in_maps = []
    for kcore in range(N_CORES):
        m = {"qneg": qneg[kcore]}
        for t in range(len(c_caps)):
            m[f"cand{t}"] = cands[t][kcore]
        in_maps.append(m)
    kw = dict(trace=True, trace_cores=list(range(N_CORES))) if trace else {}
    res = run_bass_kernel_spmd(nc, in_maps, list(range(N_CORES)), **kw)
    vals = np.concatenate([r["vals"] for r in res.results], axis=0)
    slots = np.concatenate([r["idxs"] for r in res.results], axis=0)
    return res, order, cand_idx, row_block, vals, slots


# ----------------------------------------------------------------------------
# public entry point
# ----------------------------------------------------------------------------

def kernel(pos, batch):
    pos = np.asarray(pos, dtype=np.float32)
    batch = np.asarray(batch)
    n = pos.shape[0]
    assert n % (N_CORES * P) == 0 and batch.ndim == 1 and len(batch) == n
    # single-system input (batch constant) is the supported fast path
    assert (batch == batch[0]).all()

    _, order, cand_idx, row_block, vals, slots = _run(pos)
    return _assemble(pos, order, cand_idx, row_block, vals, slots)


def profile_once(np_inputs):
    """Run once with NTFF tracing; return max per-core exec time in ns."""
    _ensure_ntff_hook()
    pos = np.asarray(np_inputs["pos"], dtype=np.float32)
    res, *_ = _run(pos, trace=True)
    print("per-core exec_time_ns:", res.exec_time_ns,
          "mean:", res.mean_exec_time_ns, "max core:", res.max_exec_time_core_id)
    if res.profile_json:
        print("ntff json:", res.profile_json)
    return res.exec_time_ns


def _ensure_ntff_hook():
    """The agent image's antenv lacks axon_hooks; shim it so trace=True works."""
    import sys
    import types
    if "antenv.axon_hooks" not in sys.modules:
        mod = types.ModuleType("antenv.axon_hooks")
        mod._hook = None
        mod.set_axon_ntff_profile_hook = lambda h: setattr(mod, "_hook", h)
        mod.get_axon_ntff_profile_hook = lambda: mod._hook
        sys.modules["antenv.axon_hooks"] = mod
        import antenv
        antenv.axon_hooks = mod
    mod = sys.modules["antenv.axon_hooks"]
    if mod.get_axon_ntff_profile_hook() is None:
        from trn_agent_boot.trn_boot import _ntff_profile_via_ctypes
        mod.set_axon_ntff_profile_hook(
            _ntff_profile_via_ctypes("/opt/axon/libaxon_pjrt.so"))


def _assemble(pos, order, cand_idx, row_block, vals, slots):
    """Host epilogue: slots -> atom ids, cutoff, unsort, edge outputs."""
    n = pos.shape[0]
    d2 = -vals                                       # ascending per row, exact
    dst_s = cand_idx[row_block[:, None], slots.astype(np.int64)]      # [n,K]

    # restore (d2, atom-index) lexicographic order for any exact ties
    ordk = np.lexsort((dst_s, d2), axis=1)
    d2 = np.take_along_axis(d2, ordk, axis=1)
    dst_s = np.take_along_axis(dst_s, ordk, axis=1)

    valid = d2 <= CUTOFF2
    src_orig = order[:, None]                        # original atom id per row
    dst_s = np.where(valid, dst_s, src_orig)         # pad -> self loop

    # un-sort rows back to original atom order
    dst = np.empty((n, K), dtype=np.int64)
    dst[order] = dst_s

    src = np.repeat(np.arange(n, dtype=np.int32), K)
    dst = dst.ravel().astype(np.int32)
    edge_index = np.stack([src, dst]).astype(np.int32)

    edge_vec = pos[src] - pos[dst]                   # raw, unwrapped
    mask = src != dst
    v2 = (edge_vec * edge_vec).sum(axis=-1)
    safe = np.where(mask, v2, np.float32(1.0))
    edge_weight = np.where(mask, np.sqrt(safe), np.float32(0.0)).astype(np.float32)
    return edge_index, edge_weight, edge_vec


# revision 29
# speedup vs baseline: 1.3980x; 1.0162x over previous
"""Periodic radius-graph KNN (minimum-image, K=32) on 8 Trainium2 cores.

Strategy (data-parallel neighbor-list build):
  * Host: sort atoms by spatial cell (8x8x8 grid of 6.25 A cells, box 50 A);
    split the sorted order into blocks of 8 atoms; for each block build a
    candidate list (atoms within cutoff of the block's bounding box, found
    via the cell grid with periodic wrap).  Blocks are then permuted so
    that blocks with similar candidate counts land in the same tile slot
    on every core -- each of the 8 tile slots gets its own compile-time
    candidate capacity C_t (the per-slot max), so DVE/DMA work scales with
    the mean candidate count instead of the global max.
  * Device (SPMD over 8 cores, 1024 atoms each): for every 128-row tile,
    broadcast each 8-row block's candidate coordinate planes across its
    partitions (DMA with partition-broadcast access pattern), compute the
    exact minimum-image squared distance
        d2 = ((mx^2 + my^2) + mz^2),  m_c = min(|dx_c|, 50 - |dx_c|)
    bit-identical to the fp32 reference, and extract the 32 smallest d2
    (with candidate indices) per row via 4 rounds of the DVE max8 /
    max_index / match_replace instructions on the negated keys.
  * Host: map candidate slots back to atom indices, apply the cutoff,
    restore original atom order, and assemble edge_index / edge_weight /
    edge_vec exactly as the reference does.

The fp32 identity min(|dx|, 50-|dx|) reproduces jnp's
`diff - round(diff/box)*box` bit-exactly for box=50 and coords in [0, 50):
the winning branch of the min is always exactly representable, so the
selection and ordering of neighbors match the reference to the last ulp.
"""

from contextlib import ExitStack

import numpy as np

import concourse.bass as bass
import concourse.tile as tile
from concourse import bacc, mybir
from concourse.bass_utils import run_bass_kernel_spmd

N_CORES = 8
K = 32
BOX = 50.0
CELL = 6.25           # 8 cells per dimension
GRID = 8
CUTOFF2 = np.float32(36.0)
BLK = 4               # rows per candidate block
P = 128               # partitions per tile
PAD_COORD = np.float32(1.0e3)   # padding slot coordinate -> d2 ~ 2.7e6 >> 36


# ----------------------------------------------------------------------------
# host-side preprocessing
# ----------------------------------------------------------------------------

def _build_blocks(pos):
    """Sort atoms by cell; per-BLK-row-block candidate index lists."""
    n = pos.shape[0]
    cell = np.minimum((pos // np.float32(CELL)).astype(np.int64), GRID - 1)
    cid = (cell[:, 0] * GRID + cell[:, 1]) * GRID + cell[:, 2]
    order = np.argsort(cid, kind="stable").astype(np.int64)
    pos_s = pos[order]
    cid_s = cid[order]

    atoms_by_cell = [[] for _ in range(GRID ** 3)]
    for i, c in enumerate(cid):
        atoms_by_cell[c].append(i)
    atoms_by_cell = [np.asarray(a, dtype=np.int64) for a in atoms_by_cell]

    n_blocks = n // BLK
    posd = pos.astype(np.float64)
    cand_lists = []
    for blk in range(n_blocks):
        rows = pos_s[blk * BLK:(blk + 1) * BLK].astype(np.float64)
        cells = np.unique(cid_s[blk * BLK:(blk + 1) * BLK])
        seen = set()
        for cc in cells:
            ca, rem = divmod(int(cc), GRID * GRID)
            cb, ccz = divmod(rem, GRID)
            for da in (-1, 0, 1):
                for db in (-1, 0, 1):
                    for dc in (-1, 0, 1):
                        seen.add(
                            (((ca + da) % GRID) * GRID + ((cb + db) % GRID)) * GRID
                            + ((ccz + dc) % GRID))
        seen = [c for c in sorted(seen) if len(atoms_by_cell[c])]
        cand = (np.concatenate([atoms_by_cell[c] for c in seen])
                if seen else np.empty(0, np.int64))
        # exact periodic distance from candidate to block bounding box
        lo = rows.min(axis=0)
        hi = rows.max(axis=0)
        p = posd[cand]
        d = np.zeros(len(cand))
        for k in range(3):
            best = None
            for sh in (-BOX, 0.0, BOX):
                x = p[:, k] + sh
                dd = np.abs(x - np.clip(x, lo[k], hi[k]))
                best = dd if best is None else np.minimum(best, dd)
            d += best * best
        keep = cand[d <= 36.0 + 1e-3]
        cand_lists.append(keep.astype(np.int64))
    return order, cand_lists


def _plan(pos):
    """Full host plan: row permutation, per-slot capacities, DRAM arrays."""
    n = pos.shape[0]
    order0, cand_lists = _build_blocks(pos)
    n_blocks = len(cand_lists)
    ntile = n // (N_CORES * P)
    blk_per_tile = P // BLK
    blocks_per_slot = N_CORES * blk_per_tile

    counts = np.array([len(c) for c in cand_lists])
    # blocks whose bbox (+)6A shell never crosses a box face need no periodic
    # wrap at all: every |dx| < 25, so d2 == adx^2 exactly.  Group them into
    # dedicated tile slots that skip the min/(adx-50)^2 stage on the DVE.
    pos_sn = pos[order0].reshape(n_blocks, BLK, 3)
    blo = pos_sn.min(axis=1)
    bhi = pos_sn.max(axis=1)
    nowrap = ((blo > 6.001) & (bhi < 43.999)).all(axis=1)
    nw_idx = np.nonzero(nowrap)[0]
    wr_idx = np.nonzero(~nowrap)[0]
    n_fast = len(nw_idx) // blocks_per_slot            # full no-wrap slots
    nw_sorted = nw_idx[np.argsort(counts[nw_idx], kind="stable")]
    rest = np.concatenate([nw_sorted[n_fast * blocks_per_slot:], wr_idx])
    rest = rest[np.argsort(counts[rest], kind="stable")]
    grps = [nw_sorted[s * blocks_per_slot:(s + 1) * blocks_per_slot]
            for s in range(n_fast)]
    flags = [True] * n_fast
    for s in range(ntile - n_fast):
        grps.append(rest[s * blocks_per_slot:(s + 1) * blocks_per_slot])
        flags.append(False)
    # order slots small, large, small, large ... (by cap) so the DMA/ACT
    # pipeline stays ahead of the DVE with minimal warm-up and tail stalls
    caps0 = [int(counts[g].max()) for g in grps]
    by_cap = np.argsort(caps0, kind="stable")
    interleave = []
    lo, hi = 0, ntile - 1
    while lo <= hi:
        interleave.append(by_cap[lo])
        if hi != lo:
            interleave.append(by_cap[hi])
        lo += 1
        hi -= 1
    grps = [grps[i] for i in interleave]
    flags = tuple(flags[i] for i in interleave)
    c_caps = []
    assign = np.empty((N_CORES, ntile, blk_per_tile), dtype=np.int64)
    for s, grp in enumerate(grps):
        cmax = max(40, int(counts[grp].max()))
        c_caps.append(-(-cmax // 8) * 8)
        assign[:, s, :] = grp.reshape(N_CORES, blk_per_tile)

    # new row order: core-major, then slot, then block, then row-in-block
    block_rows = np.arange(n).reshape(n_blocks, BLK)
    new_rows = block_rows[assign.reshape(-1)].reshape(-1)   # sorted-row indices
    order = order0[new_rows]                                # original atom ids

    pos_s = pos[order]
    # packed per-core qneg: [core][128, 3*ntile]  (col 3t+c = -coord c, tile t)
    qneg = np.empty((N_CORES, P, 3 * ntile), dtype=np.float32)
    for kcore in range(N_CORES):
        slab = -pos_s[kcore * ntile * P:(kcore + 1) * ntile * P]
        for t in range(ntile):
            qneg[kcore, :, 3 * t:3 * t + 3] = slab[t * P:(t + 1) * P]

    # per-slot candidate planes, pre-replicated across each block's rows so the
    # device loads one contiguous [128, 3C] tile per slot (fast DMA, 1 issue
    # per engine) -- list over slots: [N_CORES, P, 3*C_s] f32
    cands = []
    for s in range(ntile):
        C = c_caps[s]
        cx = np.full((N_CORES, blk_per_tile, 3 * C), PAD_COORD, dtype=np.float32)
        for kcore in range(N_CORES):
            for b in range(blk_per_tile):
                cl = cand_lists[assign[kcore, s, b]]
                plane = np.full((3, C), PAD_COORD, dtype=np.float32)
                plane[:, :len(cl)] = pos[cl].T
                cx[kcore, b] = plane.reshape(-1)
        rep = np.broadcast_to(cx[:, :, None, :],
                              (N_CORES, blk_per_tile, BLK, 3 * C))
        cands.append(np.ascontiguousarray(rep).reshape(N_CORES, P, 3 * C))

    cand_idx = np.full((n_blocks, max(c_caps)), -1, dtype=np.int64)
    for b, cl in enumerate(cand_lists):
        cand_idx[b, :len(cl)] = cl
    # block id (in original cell-sorted block numbering) for each new row
    row_block = assign.reshape(-1).repeat(BLK)
    return order, tuple(c_caps), flags, qneg, cands, cand_idx, row_block


# ----------------------------------------------------------------------------
# device kernel (built once per (rows_per_core, c_caps) shape)
# ----------------------------------------------------------------------------

def _build_nc(rows_per_core, c_caps, flags):
    ntile = rows_per_core // P
    blk_per_tile = P // BLK
    assert len(c_caps) == ntile

    nc = bacc.Bacc("TRN2", target_bir_lowering=False, debug=False,
                   enable_asserts=False, num_devices=N_CORES)
    qneg_d = nc.dram_tensor("qneg", [P, 3 * ntile], mybir.dt.float32,
                            kind="ExternalInput").ap()
    cand_d = [nc.dram_tensor(f"cand{t}", [P, 3 * c_caps[t]],
                             mybir.dt.float32, kind="ExternalInput").ap()
              for t in range(ntile)]
    vals_d = nc.dram_tensor("vals", [rows_per_core, K], mybir.dt.float32,
                            kind="ExternalOutput").ap()
    idxs_d = nc.dram_tensor("idxs", [rows_per_core, K], mybir.dt.uint32,
                            kind="ExternalOutput").ap()

    with tile.TileContext(nc) as tc, ExitStack() as ctx:
        pool = ctx.enter_context(tc.tile_pool(name="work", bufs=4))
        opool = ctx.enter_context(tc.tile_pool(name="outs", bufs=ntile))
        cpool = ctx.enter_context(tc.tile_pool(name="consts", bufs=1))
        out_tiles = []

        bias_m50 = cpool.tile([P, 1], mybir.dt.float32)
        nc.gpsimd.memset(bias_m50[:], -50.0)
        bias_0 = cpool.tile([P, 1], mybir.dt.float32)
        nc.gpsimd.memset(bias_0[:], 0.0)
        qneg = cpool.tile([P, 3 * ntile], mybir.dt.float32)
        nc.sync.dma_start(qneg[:], qneg_d[:])
        # warm the ACT table set while the first candidate DMA is in flight
        warm = cpool.tile([P, 8], mybir.dt.float32)
        nc.gpsimd.memset(warm[:], 1.0)
        nc.scalar.activation(warm[:], warm[:], mybir.ActivationFunctionType.Abs,
                             bias=bias_0[:], scale=1.0)
        nc.scalar.activation(warm[:], warm[:],
                             mybir.ActivationFunctionType.Square, bias=bias_0[:])

        for t in range(ntile):
            C = c_caps[t]
            xj = pool.tile([P, 3 * C], mybir.dt.float32, tag="xj")
            # 4 issues -> 4 parallel HWDGE engines
            for q in range(4):
                nc.sync.dma_start(xj[32 * q:32 * (q + 1), :],
                                  cand_d[t][32 * q:32 * (q + 1), :])

            # adx = |xj - xi| ; exact fp32 (fma single-rounding == plain sub)
            adx = pool.tile([P, 3 * C], mybir.dt.float32, tag="adx")
            for c in range(3):
                nc.scalar.activation(
                    adx[:, C * c:C * (c + 1)], xj[:, C * c:C * (c + 1)],
                    mybir.ActivationFunctionType.Abs,
                    bias=qneg[:, 3 * t + c:3 * t + c + 1], scale=1.0)

            # msq_c = min(adx^2, (adx-50)^2) == wrapped_diff^2, exactly.
            # No-wrap slots (block bbox + cutoff inside the box) have every
            # |dx| < 25, so adx^2 already is the wrapped square -- skip the
            # (adx-50)^2 plane and the DVE min entirely.
            sq1 = pool.tile([P, 3 * C], mybir.dt.float32, tag="xj")  # reuse xj buf
            nc.scalar.activation(sq1[:], adx[:],
                                 mybir.ActivationFunctionType.Square,
                                 bias=bias_0[:])
            if flags[t]:
                msq = sq1
            else:
                sq2 = pool.tile([P, 3 * C], mybir.dt.float32, tag="sq2")
                nc.scalar.activation(sq2[:], adx[:],
                                     mybir.ActivationFunctionType.Square,
                                     bias=bias_m50[:])
                msq = sq2
                nc.vector.tensor_tensor(msq[:], sq1[:], sq2[:],
                                        mybir.AluOpType.min)

            # key = -d2 = (-msq_x - msq_y) - msq_z  (negated left-to-right sum)
            key = pool.tile([P, C], mybir.dt.float32, tag="key")
            nc.vector.scalar_tensor_tensor(
                key[:], msq[:, 0:C], -1.0, msq[:, C:2 * C],
                mybir.AluOpType.mult, mybir.AluOpType.subtract)
            nc.vector.tensor_tensor(key[:], key[:], msq[:, 2 * C:3 * C],
                                    mybir.AluOpType.subtract)

            # 4 rounds of 8-way extraction = top-32 ascending d2
            out = opool.tile([P, 2 * K], mybir.dt.uint32, tag="out")
            vals = out[:, 0:K].bitcast(mybir.dt.float32)
            idxs = out[:, K:2 * K]
            for r in range(K // 8):
                v8 = vals[:, 8 * r:8 * (r + 1)]
                nc.vector.max(v8, key[:])
                nc.vector.max_index(idxs[:, 8 * r:8 * (r + 1)], v8, key[:])
                if r != K // 8 - 1:
                    nc.vector.match_replace(key[:], v8, key[:], -3.0e38)

            # outputs on the gpsimd queue (which carries nothing else per-tile,
            # so no later tile's input path ever blocks on them)
            nc.gpsimd.dma_start(vals_d[t * P:(t + 1) * P, :], vals[:])
            nc.gpsimd.dma_start(idxs_d[t * P:(t + 1) * P, :], idxs[:])
    nc.compile()
    return nc


_NC_CACHE = {}


def _get_nc(rows_per_core, c_caps, flags):
    key = (rows_per_core, c_caps, flags)
    if key not in _NC_CACHE:
        _NC_CACHE[key] = _build_nc(rows_per_core, c_caps, flags)
    return _NC_CACHE[key]


def _run(pos, trace=False):
    order, c_caps, flags, qneg, cands, cand_idx, row_block = _plan(pos)
    n = pos.shape[0]
    rows_per_core = n // N_CORES
    nc = _get_nc(rows_per_core, c_caps, flags)
    in_maps = []
    for kcore in range(N_CORES):
        m = {"qneg": qneg[kcore]}
        for t in range(len(c_caps)):
            m[f"cand{t}"] = cands[t][kcore]
        in_maps.append(m)
    kw = dict(trace=True, trace_cores=list(range(N_CORES))) if trace else {}
    res = run_bass_kernel_spmd(nc, in_maps, list(range(N_CORES)), **kw)
    vals = np.concatenate([r["vals"] for r in res.results], axis=0)
    slots = np.concatenate([r["idxs"] for r in res.results], axis=0)
    return res, order, cand_idx, row_block, vals, slots


# ----------------------------------------------------------------------------
# public entry point
# ----------------------------------------------------------------------------

def kernel(pos, batch):
    pos = np.asarray(pos, dtype=np.float32)
    batch = np.asarray(batch)
    n = pos.shape[0]
    assert n % (N_CORES * P) == 0 and batch.ndim == 1 and len(batch) == n
    # single-system input (batch constant) is the supported fast path
    assert (batch == batch[0]).all()

    _, order, cand_idx, row_block, vals, slots = _run(pos)
    return _assemble(pos, order, cand_idx, row_block, vals, slots)


def profile_once(np_inputs):
    """Run once with NTFF tracing; return max per-core exec time in ns."""
    _ensure_ntff_hook()
    pos = np.asarray(np_inputs["pos"], dtype=np.float32)
    res, *_ = _run(pos, trace=True)
    print("per-core exec_time_ns:", res.exec_time_ns,
          "mean:", res.mean_exec_time_ns, "max core:", res.max_exec_time_core_id)
    if res.profile_json:
        print("ntff json:", res.profile_json)
    return res.exec_time_ns


def _ensure_ntff_hook():
    """The agent image's antenv lacks axon_hooks; shim it so trace=True works."""
    import sys
    import types
    if "antenv.axon_hooks" not in sys.modules:
        mod = types.ModuleType("antenv.axon_hooks")
        mod._hook = None
        mod.set_axon_ntff_profile_hook = lambda h: setattr(mod, "_hook", h)
        mod.get_axon_ntff_profile_hook = lambda: mod._hook
        sys.modules["antenv.axon_hooks"] = mod
        import antenv
        antenv.axon_hooks = mod
    mod = sys.modules["antenv.axon_hooks"]
    if mod.get_axon_ntff_profile_hook() is None:
        from trn_agent_boot.trn_boot import _ntff_profile_via_ctypes
        mod.set_axon_ntff_profile_hook(
            _ntff_profile_via_ctypes("/opt/axon/libaxon_pjrt.so"))


def _assemble(pos, order, cand_idx, row_block, vals, slots):
    """Host epilogue: slots -> atom ids, cutoff, unsort, edge outputs."""
    n = pos.shape[0]
    d2 = -vals                                       # ascending per row, exact
    dst_s = cand_idx[row_block[:, None], slots.astype(np.int64)]      # [n,K]

    # restore (d2, atom-index) lexicographic order for any exact ties
    ordk = np.lexsort((dst_s, d2), axis=1)
    d2 = np.take_along_axis(d2, ordk, axis=1)
    dst_s = np.take_along_axis(dst_s, ordk, axis=1)

    valid = d2 <= CUTOFF2
    src_orig = order[:, None]                        # original atom id per row
    dst_s = np.where(valid, dst_s, src_orig)         # pad -> self loop

    # un-sort rows back to original atom order
    dst = np.empty((n, K), dtype=np.int64)
    dst[order] = dst_s

    src = np.repeat(np.arange(n, dtype=np.int32), K)
    dst = dst.ravel().astype(np.int32)
    edge_index = np.stack([src, dst]).astype(np.int32)

    edge_vec = pos[src] - pos[dst]                   # raw, unwrapped
    mask = src != dst
    v2 = (edge_vec * edge_vec).sum(axis=-1)
    safe = np.where(mask, v2, np.float32(1.0))
    edge_weight = np.where(mask, np.sqrt(safe), np.float32(0.0)).astype(np.float32)
    return edge_index, edge_weight, edge_vec


# revision 32
# speedup vs baseline: 1.4247x; 1.0191x over previous
"""Periodic radius-graph KNN (minimum-image, K=32) on 8 Trainium2 cores.

Strategy (data-parallel neighbor-list build):
  * Host: sort atoms by spatial cell (8x8x8 grid of 6.25 A cells, box 50 A);
    split the sorted order into blocks of 8 atoms; for each block build a
    candidate list (atoms within cutoff of the block's bounding box, found
    via the cell grid with periodic wrap).  Blocks are then permuted so
    that blocks with similar candidate counts land in the same tile slot
    on every core -- each of the 8 tile slots gets its own compile-time
    candidate capacity C_t (the per-slot max), so DVE/DMA work scales with
    the mean candidate count instead of the global max.
  * Device (SPMD over 8 cores, 1024 atoms each): for every 128-row tile,
    broadcast each 8-row block's candidate coordinate planes across its
    partitions (DMA with partition-broadcast access pattern), compute the
    exact minimum-image squared distance
        d2 = ((mx^2 + my^2) + mz^2),  m_c = min(|dx_c|, 50 - |dx_c|)
    bit-identical to the fp32 reference, and extract the 32 smallest d2
    (with candidate indices) per row via 4 rounds of the DVE max8 /
    max_index / match_replace instructions on the negated keys.
  * Host: map candidate slots back to atom indices, apply the cutoff,
    restore original atom order, and assemble edge_index / edge_weight /
    edge_vec exactly as the reference does.

The fp32 identity min(|dx|, 50-|dx|) reproduces jnp's
`diff - round(diff/box)*box` bit-exactly for box=50 and coords in [0, 50):
the winning branch of the min is always exactly representable, so the
selection and ordering of neighbors match the reference to the last ulp.
"""

from contextlib import ExitStack

import numpy as np

import concourse.bass as bass
import concourse.tile as tile
from concourse import bacc, mybir
from concourse.bass_utils import run_bass_kernel_spmd

N_CORES = 8
K = 32
BOX = 50.0
CELL = 6.25           # 8 cells per dimension
GRID = 8
CUTOFF2 = np.float32(36.0)
BLK = 4               # rows per candidate block
P = 128               # partitions per tile
PAD_COORD = np.float32(1.0e3)   # padding slot coordinate -> d2 ~ 2.7e6 >> 36


# ----------------------------------------------------------------------------
# host-side preprocessing
# ----------------------------------------------------------------------------

def _build_blocks(pos):
    """Sort atoms by cell; per-BLK-row-block candidate index lists."""
    n = pos.shape[0]
    cell = np.minimum((pos // np.float32(CELL)).astype(np.int64), GRID - 1)
    cid = (cell[:, 0] * GRID + cell[:, 1]) * GRID + cell[:, 2]
    order = np.argsort(cid, kind="stable").astype(np.int64)
    pos_s = pos[order]
    cid_s = cid[order]

    atoms_by_cell = [[] for _ in range(GRID ** 3)]
    for i, c in enumerate(cid):
        atoms_by_cell[c].append(i)
    atoms_by_cell = [np.asarray(a, dtype=np.int64) for a in atoms_by_cell]

    n_blocks = n // BLK
    posd = pos.astype(np.float64)
    cand_lists = []
    for blk in range(n_blocks):
        rows = pos_s[blk * BLK:(blk + 1) * BLK].astype(np.float64)
        cells = np.unique(cid_s[blk * BLK:(blk + 1) * BLK])
        seen = set()
        for cc in cells:
            ca, rem = divmod(int(cc), GRID * GRID)
            cb, ccz = divmod(rem, GRID)
            for da in (-1, 0, 1):
                for db in (-1, 0, 1):
                    for dc in (-1, 0, 1):
                        seen.add(
                            (((ca + da) % GRID) * GRID + ((cb + db) % GRID)) * GRID
                            + ((ccz + dc) % GRID))
        seen = [c for c in sorted(seen) if len(atoms_by_cell[c])]
        cand = (np.concatenate([atoms_by_cell[c] for c in seen])
                if seen else np.empty(0, np.int64))
        # exact periodic distance from candidate to block bounding box
        lo = rows.min(axis=0)
        hi = rows.max(axis=0)
        p = posd[cand]
        d = np.zeros(len(cand))
        for k in range(3):
            best = None
            for sh in (-BOX, 0.0, BOX):
                x = p[:, k] + sh
                dd = np.abs(x - np.clip(x, lo[k], hi[k]))
                best = dd if best is None else np.minimum(best, dd)
            d += best * best
        keep = cand[d <= 36.0 + 1e-3]
        cand_lists.append(keep.astype(np.int64))
    return order, cand_lists


def _plan(pos):
    """Full host plan: row permutation, per-slot capacities, DRAM arrays."""
    n = pos.shape[0]
    order0, cand_lists = _build_blocks(pos)
    n_blocks = len(cand_lists)
    ntile = n // (N_CORES * P)
    blk_per_tile = P // BLK
    blocks_per_slot = N_CORES * blk_per_tile

    counts = np.array([len(c) for c in cand_lists])
    # blocks whose bbox (+)6A shell never crosses a box face need no periodic
    # wrap at all: every |dx| < 25, so d2 == adx^2 exactly.  Group them into
    # dedicated tile slots that skip the min/(adx-50)^2 stage on the DVE.
    pos_sn = pos[order0].reshape(n_blocks, BLK, 3)
    blo = pos_sn.min(axis=1)
    bhi = pos_sn.max(axis=1)
    cross = (blo <= 6.001) | (bhi >= 43.999)       # per block, per coord
    nowrap = ~cross.any(axis=1)
    nw_idx = np.nonzero(nowrap)[0]
    wr_idx = np.nonzero(~nowrap)[0]
    n_fast = len(nw_idx) // blocks_per_slot            # full no-wrap slots
    nw_sorted = nw_idx[np.argsort(counts[nw_idx], kind="stable")]
    rest = np.concatenate([nw_sorted[n_fast * blocks_per_slot:], wr_idx])
    rest = rest[np.argsort(counts[rest], kind="stable")]
    grps = [nw_sorted[s * blocks_per_slot:(s + 1) * blocks_per_slot]
            for s in range(n_fast)]
    for s in range(ntile - n_fast):
        grps.append(rest[s * blocks_per_slot:(s + 1) * blocks_per_slot])
    # per-slot, per-coordinate wrap mask: the (adx-50)^2 / min stage is only
    # needed for coordinates where some block in the slot crosses a box face
    flags = [tuple(bool(cross[g, c].any()) for c in range(3)) for g in grps]
    # order slots small, large, small, large ... (by cap) so the DMA/ACT
    # pipeline stays ahead of the DVE with minimal warm-up and tail stalls
    caps0 = [int(counts[g].max()) for g in grps]
    by_cap = np.argsort(caps0, kind="stable")
    interleave = []
    lo, hi = 0, ntile - 1
    while lo <= hi:
        interleave.append(by_cap[lo])
        if hi != lo:
            interleave.append(by_cap[hi])
        lo += 1
        hi -= 1
    grps = [grps[i] for i in interleave]
    flags = tuple(flags[i] for i in interleave)
    c_caps = []
    assign = np.empty((N_CORES, ntile, blk_per_tile), dtype=np.int64)
    for s, grp in enumerate(grps):
        cmax = max(40, int(counts[grp].max()))
        c_caps.append(-(-cmax // 8) * 8)
        assign[:, s, :] = grp.reshape(N_CORES, blk_per_tile)

    # new row order: core-major, then slot, then block, then row-in-block
    block_rows = np.arange(n).reshape(n_blocks, BLK)
    new_rows = block_rows[assign.reshape(-1)].reshape(-1)   # sorted-row indices
    order = order0[new_rows]                                # original atom ids

    pos_s = pos[order]
    # packed per-core qneg: [core][128, 3*ntile]  (col 3t+c = -coord c, tile t)
    qneg = np.empty((N_CORES, P, 3 * ntile), dtype=np.float32)
    for kcore in range(N_CORES):
        slab = -pos_s[kcore * ntile * P:(kcore + 1) * ntile * P]
        for t in range(ntile):
            qneg[kcore, :, 3 * t:3 * t + 3] = slab[t * P:(t + 1) * P]

    # per-slot candidate planes, pre-replicated across each block's rows so the
    # device loads one contiguous [128, 3C] tile per slot (fast DMA, 1 issue
    # per engine) -- list over slots: [N_CORES, P, 3*C_s] f32
    cands = []
    for s in range(ntile):
        C = c_caps[s]
        cx = np.full((N_CORES, blk_per_tile, 3 * C), PAD_COORD, dtype=np.float32)
        for kcore in range(N_CORES):
            for b in range(blk_per_tile):
                cl = cand_lists[assign[kcore, s, b]]
                plane = np.full((3, C), PAD_COORD, dtype=np.float32)
                plane[:, :len(cl)] = pos[cl].T
                cx[kcore, b] = plane.reshape(-1)
        rep = np.broadcast_to(cx[:, :, None, :],
                              (N_CORES, blk_per_tile, BLK, 3 * C))
        cands.append(np.ascontiguousarray(rep).reshape(N_CORES, P, 3 * C))

    cand_idx = np.full((n_blocks, max(c_caps)), -1, dtype=np.int64)
    for b, cl in enumerate(cand_lists):
        cand_idx[b, :len(cl)] = cl
    # block id (in original cell-sorted block numbering) for each new row
    row_block = assign.reshape(-1).repeat(BLK)
    return order, tuple(c_caps), flags, qneg, cands, cand_idx, row_block


# ----------------------------------------------------------------------------
# device kernel (built once per (rows_per_core, c_caps) shape)
# ----------------------------------------------------------------------------

def _build_nc(rows_per_core, c_caps, flags):
    ntile = rows_per_core // P
    blk_per_tile = P // BLK
    assert len(c_caps) == ntile

    nc = bacc.Bacc("TRN2", target_bir_lowering=False, debug=False,
                   enable_asserts=False, num_devices=N_CORES)
    qneg_d = nc.dram_tensor("qneg", [P, 3 * ntile], mybir.dt.float32,
                            kind="ExternalInput").ap()
    cand_d = [nc.dram_tensor(f"cand{t}", [P, 3 * c_caps[t]],
                             mybir.dt.float32, kind="ExternalInput").ap()
              for t in range(ntile)]
    vals_d = nc.dram_tensor("vals", [rows_per_core, K], mybir.dt.float32,
                            kind="ExternalOutput").ap()
    idxs_d = nc.dram_tensor("idxs", [rows_per_core, K], mybir.dt.uint32,
                            kind="ExternalOutput").ap()

    with tile.TileContext(nc) as tc, ExitStack() as ctx:
        pool = ctx.enter_context(tc.tile_pool(name="work", bufs=4))
        opool = ctx.enter_context(tc.tile_pool(name="outs", bufs=ntile))
        cpool = ctx.enter_context(tc.tile_pool(name="consts", bufs=1))
        out_tiles = []

        bias_m50 = cpool.tile([P, 1], mybir.dt.float32)
        nc.gpsimd.memset(bias_m50[:], -50.0)
        bias_0 = cpool.tile([P, 1], mybir.dt.float32)
        nc.gpsimd.memset(bias_0[:], 0.0)
        qneg = cpool.tile([P, 3 * ntile], mybir.dt.float32)
        nc.sync.dma_start(qneg[:], qneg_d[:])
        # warm the ACT table set while the first candidate DMA is in flight
        warm = cpool.tile([P, 8], mybir.dt.float32)
        nc.gpsimd.memset(warm[:], 1.0)
        nc.scalar.activation(warm[:], warm[:], mybir.ActivationFunctionType.Abs,
                             bias=bias_0[:], scale=1.0)
        nc.scalar.activation(warm[:], warm[:],
                             mybir.ActivationFunctionType.Square, bias=bias_0[:])

        for t in range(ntile):
            C = c_caps[t]
            xj = pool.tile([P, 3 * C], mybir.dt.float32, tag="xj")
            # 4 issues -> 4 parallel HWDGE engines
            for q in range(4):
                nc.sync.dma_start(xj[32 * q:32 * (q + 1), :],
                                  cand_d[t][32 * q:32 * (q + 1), :])

            # adx = |xj - xi| ; exact fp32 (fma single-rounding == plain sub)
            adx = pool.tile([P, 3 * C], mybir.dt.float32, tag="adx")
            for c in range(3):
                nc.scalar.activation(
                    adx[:, C * c:C * (c + 1)], xj[:, C * c:C * (c + 1)],
                    mybir.ActivationFunctionType.Abs,
                    bias=qneg[:, 3 * t + c:3 * t + c + 1], scale=1.0)

            # msq_c = min(adx^2, (adx-50)^2) == wrapped_diff^2, exactly.
            # Coordinates where no block of the slot crosses a box face have
            # every in-cutoff |dx| <= 6, so adx^2 already is the wrapped
            # square -- skip the (adx-50)^2 plane and the DVE min for them.
            sq1 = pool.tile([P, 3 * C], mybir.dt.float32, tag="xj")  # reuse xj buf
            nc.scalar.activation(sq1[:], adx[:],
                                 mybir.ActivationFunctionType.Square,
                                 bias=bias_0[:])
            planes = [sq1[:, C * c:C * (c + 1)] for c in range(3)]
            if any(flags[t]):
                sq2 = pool.tile([P, 3 * C], mybir.dt.float32, tag="sq2")
                for c in range(3):
                    if not flags[t][c]:
                        continue
                    s2c = sq2[:, C * c:C * (c + 1)]
                    nc.scalar.activation(s2c, adx[:, C * c:C * (c + 1)],
                                         mybir.ActivationFunctionType.Square,
                                         bias=bias_m50[:])
                    nc.vector.tensor_tensor(s2c, planes[c], s2c,
                                            mybir.AluOpType.min)
                    planes[c] = s2c

            # key = -d2 = (-msq_x - msq_y) - msq_z  (negated left-to-right sum)
            key = pool.tile([P, C], mybir.dt.float32, tag="key")
            nc.vector.scalar_tensor_tensor(
                key[:], planes[0], -1.0, planes[1],
                mybir.AluOpType.mult, mybir.AluOpType.subtract)
            nc.vector.tensor_tensor(key[:], key[:], planes[2],
                                    mybir.AluOpType.subtract)

            # 4 rounds of 8-way extraction = top-32 ascending d2
            out = opool.tile([P, 2 * K], mybir.dt.uint32, tag="out")
            vals = out[:, 0:K].bitcast(mybir.dt.float32)
            idxs = out[:, K:2 * K]
            for r in range(K // 8):
                v8 = vals[:, 8 * r:8 * (r + 1)]
                nc.vector.max(v8, key[:])
                nc.vector.max_index(idxs[:, 8 * r:8 * (r + 1)], v8, key[:])
                if r != K // 8 - 1:
                    nc.vector.match_replace(key[:], v8, key[:], -3.0e38)

            # outputs on the gpsimd queue (which carries nothing else per-tile,
            # so no later tile's input path ever blocks on them)
            nc.gpsimd.dma_start(vals_d[t * P:(t + 1) * P, :], vals[:])
            nc.gpsimd.dma_start(idxs_d[t * P:(t + 1) * P, :], idxs[:])
    nc.compile()
    return nc


_NC_CACHE = {}


def _get_nc(rows_per_core, c_caps, flags):
    key = (rows_per_core, c_caps, flags)
    if key not in _NC_CACHE:
        _NC_CACHE[key] = _build_nc(rows_per_core, c_caps, flags)
    return _NC_CACHE[key]


def _run(pos, trace=False):
    order, c_caps, flags, qneg, cands, cand_idx, row_block = _plan(pos)
    n = pos.shape[0]
    rows_per_core = n // N_CORES
    nc = _get_nc(rows_per_core, c_caps, flags)
    in_maps = []
    for kcore in range(N_CORES):
        m = {"qneg": qneg[kcore]}
        for t in range(len(c_caps)):
            m[f"cand{t}"] = cands[t][kcore]
        in_maps.append(m)
    kw = dict(trace=True, trace_cores=list(range(N_CORES))) if trace else {}
    res = run_bass_kernel_spmd(nc, in_maps, list(range(N_CORES)), **kw)
    vals = np.concatenate([r["vals"] for r in res.results], axis=0)
    slots = np.concatenate([r["idxs"] for r in res.results], axis=0)
    return res, order, cand_idx, row_block, vals, slots


# ----------------------------------------------------------------------------
# public entry point
# ----------------------------------------------------------------------------

def kernel(pos, batch):
    pos = np.asarray(pos, dtype=np.float32)
    batch = np.asarray(batch)
    n = pos.shape[0]
    assert n % (N_CORES * P) == 0 and batch.ndim == 1 and len(batch) == n
    # single-system input (batch constant) is the supported fast path
    assert (batch == batch[0]).all()

    _, order, cand_idx, row_block, vals, slots = _run(pos)
    return _assemble(pos, order, cand_idx, row_block, vals, slots)


def profile_once(np_inputs):
    """Run once with NTFF tracing; return max per-core exec time in ns."""
    _ensure_ntff_hook()
    pos = np.asarray(np_inputs["pos"], dtype=np.float32)
    res, *_ = _run(pos, trace=True)
    print("per-core exec_time_ns:", res.exec_time_ns,
          "mean:", res.mean_exec_time_ns, "max core:", res.max_exec_time_core_id)
    if res.profile_json:
        print("ntff json:", res.profile_json)
    return res.exec_time_ns


def _ensure_ntff_hook():
    """The agent image's antenv lacks axon_hooks; shim it so trace=True works."""
    import sys
    import types
    if "antenv.axon_hooks" not in sys.modules:
        mod = types.ModuleType("antenv.axon_hooks")
        mod._hook = None
        mod.set_axon_ntff_profile_hook = lambda h: setattr(mod, "_hook", h)
        mod.get_axon_ntff_profile_hook = lambda: mod._hook
        sys.modules["antenv.axon_hooks"] = mod
        import antenv
        antenv.axon_hooks = mod
    mod = sys.modules["antenv.axon_hooks"]
    if mod.get_axon_ntff_profile_hook() is None:
        from trn_agent_boot.trn_boot import _ntff_profile_via_ctypes
        mod.set_axon_ntff_profile_hook(
            _ntff_profile_via_ctypes("/opt/axon/libaxon_pjrt.so"))


def _assemble(pos, order, cand_idx, row_block, vals, slots):
    """Host epilogue: slots -> atom ids, cutoff, unsort, edge outputs."""
    n = pos.shape[0]
    d2 = -vals                                       # ascending per row, exact
    dst_s = cand_idx[row_block[:, None], slots.astype(np.int64)]      # [n,K]

    # restore (d2, atom-index) lexicographic order for any exact ties
    ordk = np.lexsort((dst_s, d2), axis=1)
    d2 = np.take_along_axis(d2, ordk, axis=1)
    dst_s = np.take_along_axis(dst_s, ordk, axis=1)

    valid = d2 <= CUTOFF2
    src_orig = order[:, None]                        # original atom id per row
    dst_s = np.where(valid, dst_s, src_orig)         # pad -> self loop

    # un-sort rows back to original atom order
    dst = np.empty((n, K), dtype=np.int64)
    dst[order] = dst_s

    src = np.repeat(np.arange(n, dtype=np.int32), K)
    dst = dst.ravel().astype(np.int32)
    edge_index = np.stack([src, dst]).astype(np.int32)

    edge_vec = pos[src] - pos[dst]                   # raw, unwrapped
    mask = src != dst
    v2 = (edge_vec * edge_vec).sum(axis=-1)
    safe = np.where(mask, v2, np.float32(1.0))
    edge_weight = np.where(mask, np.sqrt(safe), np.float32(0.0)).astype(np.float32)
    return edge_index, edge_weight, edge_vec
